# revision 14
# baseline (speedup 1.0000x reference)
"""GPT-2 (L=12, B=4, T=1024, C=768, H=12, V=50257) forward pass on 8 TRN2
NeuronCores.

Sharding: 8 cores = 4 sequences x 2 cores.  Core (b, s) owns the
INTERLEAVED query blocks {s, s+2, s+4, s+6} (128 rows each) of sequence b,
so both cores of a pair need the same causal chunk profile (2,4,6,8) per
local block and 37.5% of the S/AV/exp work is statically skipped.
Per-layer pairwise AllGather of (K^T, V-ext); gathered keys are assembled
in natural order.  Causal structure: key chunk j covers only the query
blocks that can see it (suffix of the local blocks); the two top chunks of
each block get a multiplicative [128x128] mask (tril / zeros / ones, a
per-core input).

Row groups g0 = blocks {0,1}, g1 = {2,3} are pipelined: attention of g1
(ACT-engine heavy) overlaps proj/LN2/MLP of g0 (PE heavy).

Compute dtype: bf16 matmuls with fp32 PSUM accumulation; fp32 residual
stream, layernorm stats, and softmax accumulation; bf16 logits.
"""

import numpy as np
import ml_dtypes

import concourse.bass as bass
import concourse.tile as tile
import concourse.mybir as mybir
from concourse import bacc, bass_utils
from concourse.masks import make_identity

F32 = mybir.dt.float32
BF16 = mybir.dt.bfloat16
AF = mybir.ActivationFunctionType
OP = mybir.AluOpType

L, B, T, C, H, V = 12, 4, 1024, 768, 12, 50257
D = C // H            # 64
FF = 4 * C            # 3072
R = 512               # rows per core
NC_ = 8               # cores
KO = C // 128         # 6
RT = R // 128         # 4 row tiles (local blocks)
NVS = 99              # vocab slices of 512
VP = NVS * 512        # 50688 padded vocab
SCALE = 1.0 / float(np.sqrt(D))
EPS = 1e-5

_BUILD_CACHE = {}


def _build_nc(sim=False):
    nc = bacc.Bacc("TRN2", target_bir_lowering=False, debug=False,
                   num_devices=NC_)

    # ---- I/O ----
    x0_d = nc.dram_tensor("x0", [R, C], F32, kind="ExternalInput")
    mask_d = nc.dram_tensor("maskt", [8, 128, 128], BF16, kind="ExternalInput")
    wq_d = nc.dram_tensor("wq", [L, KO, 128, C], BF16, kind="ExternalInput")
    wk_d = nc.dram_tensor("wk", [L, KO, 128, C], BF16, kind="ExternalInput")
    wv_d = nc.dram_tensor("wv", [L, 128, KO, C], BF16, kind="ExternalInput")
    wo_d = nc.dram_tensor("wo", [L, 128, KO, C], BF16, kind="ExternalInput")
    w1_d = nc.dram_tensor("w1", [L, FF // 128, 128, C], BF16, kind="ExternalInput")
    w2_d = nc.dram_tensor("w2", [L, FF, C], BF16, kind="ExternalInput")
    bq_d = nc.dram_tensor("bq", [L, C], F32, kind="ExternalInput")
    bk_d = nc.dram_tensor("bk", [L, C], F32, kind="ExternalInput")
    bv_d = nc.dram_tensor("bv", [L, C], BF16, kind="ExternalInput")
    bo_d = nc.dram_tensor("bo", [L, C], BF16, kind="ExternalInput")
    b1_d = nc.dram_tensor("b1", [L, FF], F32, kind="ExternalInput")
    b2_d = nc.dram_tensor("b2", [L, C], BF16, kind="ExternalInput")
    g1_d = nc.dram_tensor("ln1g", [L, C], F32, kind="ExternalInput")
    h1_d = nc.dram_tensor("ln1b", [L, C], F32, kind="ExternalInput")
    g2_d = nc.dram_tensor("ln2g", [L, C], F32, kind="ExternalInput")
    h2_d = nc.dram_tensor("ln2b", [L, C], F32, kind="ExternalInput")
    gf_d = nc.dram_tensor("lnfg", [C], F32, kind="ExternalInput")
    hf_d = nc.dram_tensor("lnfb", [C], F32, kind="ExternalInput")
    hw_d = nc.dram_tensor("headw", [128, KO, VP], BF16, kind="ExternalInput")
    out_d = nc.dram_tensor("logits", [R, V], BF16, kind="ExternalOutput")

    KVLEN = C * R
    VLEN = RT * H * 65 * 128
    with tile.TileContext(nc) as tc:
        with (
            tc.tile_pool(name="const", bufs=1) as const,
            tc.tile_pool(name="persist", bufs=1) as persist,
            tc.tile_pool(name="wsmall", bufs=3) as wsmall,
            tc.tile_pool(name="wmed", bufs=2) as wmed,
            tc.tile_pool(name="wop", bufs=1) as wop,
            tc.tile_pool(name="wc", bufs=1) as wcp,
            tc.tile_pool(name="cvec", bufs=2) as cvec,
            tc.tile_pool(name="act1", bufs=2) as act1,
            tc.tile_pool(name="attn", bufs=1) as attnp,
            tc.tile_pool(name="pt", bufs=6) as ptp,
            tc.tile_pool(name="gt", bufs=1) as gtp,
            tc.tile_pool(name="st", bufs=3) as stp,
            tc.tile_pool(name="ps", bufs=3, space="PSUM") as ps,
            tc.tile_pool(name="psy", bufs=3, space="PSUM") as psy,
            tc.tile_pool(name="psf", bufs=1, space="PSUM") as psf,
            tc.tile_pool(name="dram", bufs=2, space="DRAM") as dramp,
        ):
            ident = const.tile([128, 128], BF16)
            make_identity(nc, ident[:])
            eps_t = const.tile([128, 1], F32)
            nc.vector.memset(eps_t[:], EPS)
            ones_r = const.tile([1, 128], BF16)
            nc.vector.memset(ones_r[:], 1.0)
            mask_sb = const.tile([128, 8, 128], BF16)
            nc.sync.dma_start(mask_sb[:], mask_d.ap().rearrange("j k q -> k j q"))

            # residual stream, fp32, [128, rt, C]; rt = local block
            x_sb = persist.tile([128, RT, C], F32)
            nc.sync.dma_start(
                x_sb[:], x0_d.ap().rearrange("(o p) c -> p o c", p=128))

            def layernorm_T(gT, bT, rts, tag):
                """LN(x[rts]) -> bf16, transposed into hT [128, KO, 128*len(rts)].

                gT/bT are [128, KO] partition-form gain/bias, applied after
                the transpose (feature dim lands on partitions there)."""
                hT = attnp.tile([128, KO, 128 * len(rts)], BF16, tag=tag)
                for ri, rt in enumerate(rts):
                    xs = x_sb[:, rt, :]
                    stats = act1.tile([128, 3, 6], F32, tag=f"st{tag}")
                    xs3 = xs.rearrange("p (s d) -> p s d", s=3)
                    for s in range(3):
                        nc.vector.bn_stats(stats[:, s, :], xs3[:, s, :])
                    mv = act1.tile([128, 2], F32, tag=f"mv{tag}")
                    nc.vector.bn_aggr(mv[:], stats[:])
                    std = act1.tile([128, 1], F32, tag=f"sd{tag}")
                    nc.scalar.activation(std[:], mv[:, 1:2], AF.Sqrt,
                                         bias=eps_t[:])
                    nc.vector.reciprocal(std[:], std[:])
                    hnb = act1.tile([128, C], BF16, tag=f"hb{tag}")
                    nc.vector.tensor_scalar(hnb[:], xs, mv[:, 0:1], std[:],
                                            op0=OP.subtract, op1=OP.mult)
                    for ko in range(KO):
                        pt_ = ps.tile([128, 512], BF16, tag="acc")
                        nc.tensor.transpose(pt_[:, :128],
                                            hnb[:, ko * 128:(ko + 1) * 128],
                                            ident[:])
                        nc.vector.tensor_scalar(
                            hT[:, ko, ri * 128:(ri + 1) * 128], pt_[:, :128],
                            gT[:, ko:ko + 1], bT[:, ko:ko + 1],
                            op0=OP.mult, op1=OP.add)
                return hT

            for l in range(L):
                # --- per-layer constant vectors ---
                bqT = cvec.tile([128, KO], F32, tag="bqT")
                nc.sync.dma_start(bqT[:], bq_d.ap()[l].rearrange("(o p) -> p o", p=128))
                bkT = cvec.tile([128, KO], F32, tag="bkT")
                nc.sync.dma_start(bkT[:], bk_d.ap()[l].rearrange("(o p) -> p o", p=128))
                b1T = cvec.tile([128, FF // 128], F32, tag="b1T")
                nc.sync.dma_start(b1T[:], b1_d.ap()[l].rearrange("(o p) -> p o", p=128))
                g1T = cvec.tile([128, KO], F32, tag="g1T")
                nc.sync.dma_start(g1T[:], g1_d.ap()[l].rearrange("(o p) -> p o", p=128))
                h1T = cvec.tile([128, KO], F32, tag="h1T")
                nc.sync.dma_start(h1T[:], h1_d.ap()[l].rearrange("(o p) -> p o", p=128))
                g2T = cvec.tile([128, KO], F32, tag="g2T")
                nc.sync.dma_start(g2T[:], g2_d.ap()[l].rearrange("(o p) -> p o", p=128))
                h2T = cvec.tile([128, KO], F32, tag="h2T")
                nc.sync.dma_start(h2T[:], h2_d.ap()[l].rearrange("(o p) -> p o", p=128))
                bv_t = cvec.tile([1, C], BF16, tag="bv")
                nc.sync.dma_start(bv_t[:], bv_d.ap()[l][None, :])
                bo_t = cvec.tile([1, C], BF16, tag="bo")
                nc.sync.dma_start(bo_t[:], bo_d.ap()[l][None, :])
                b2_t = cvec.tile([1, C], BF16, tag="b2")
                nc.sync.dma_start(b2_t[:], b2_d.ap()[l][None, :])
                # prefetch the full-layer wo / w2 caches early
                won = wop.tile([128, KO, C], BF16, tag="wo")
                nc.sync.dma_start(won[:], wo_d.ap()[l])
                w2c = wcp.tile([128, FF // 128, C], BF16, tag="w2c")
                nc.sync.dma_start(
                    w2c[:], w2_d.ap()[l].rearrange("(m p) c -> p m c", p=128))

                # --- LN1 -> hT ---
                hT = layernorm_T(g1T, h1T, (0, 1, 2, 3), "ht")

                # --- kT = (h Wk)^T + bk ---  [128, KO, R]
                kT = attnp.tile([128, KO, R], BF16, tag="kt")
                for m in range(KO):
                    wkm = wsmall.tile([128, KO, 128], BF16, tag="wk")
                    nc.sync.dma_start(
                        wkm[:], wk_d.ap()[l, m].rearrange("p (ko j) -> p ko j", ko=KO))
                    acc = ps.tile([128, 512], F32, tag="acc")
                    for k in range(KO):
                        nc.tensor.matmul(acc[:], wkm[:, k, :], hT[:, k, :],
                                         start=(k == 0), stop=(k == KO - 1))
                    nc.vector.tensor_scalar_add(kT[:, m, :], acc[:],
                                                bkT[:, m:m + 1])
                k_in = dramp.tile([KVLEN], BF16, tag="kin")
                nc.sync.dma_start(
                    k_in[:].rearrange("(ko p r) -> p ko r", p=128, r=R),
                    kT[:])
                k_out = dramp.tile([2, KVLEN], BF16, tag="kout")
                if sim:
                    for rank in range(2):
                        nc.sync.dma_start(k_out[rank], k_in[:])
                else:
                    nc.gpsimd.collective_compute(
                        "AllGather", OP.bypass,
                        replica_groups=[[0, 1], [2, 3], [4, 5], [6, 7]],
                        ins=[k_in[:].opt()], outs=[k_out[:].opt()])

                # --- v = h Wv + bv ---  stored vext-shaped [128, RT, H, 65]
                v_sb = attnp.tile([128, RT, H, 65], BF16, tag="v")
                nc.vector.memset(v_sb[:, :, :, 64:65], 1.0)
                for nch, (n0, nw) in enumerate(((0, 512), (512, 256))):
                    wvn = wmed.tile([128, KO, 512], BF16, tag="wv")
                    nc.sync.dma_start(wvn[:, :, :nw],
                                      wv_d.ap()[l][:, :, n0:n0 + nw])
                    h0 = n0 // 64
                    for rt in range(RT):
                        acc = ps.tile([128, 512], F32, tag="acc")
                        for k in range(KO):
                            nc.tensor.matmul(
                                acc[:, :nw], hT[:, k, rt * 128:(rt + 1) * 128],
                                wvn[:, k, :nw],
                                start=(k == 0), stop=False)
                        nc.tensor.matmul(acc[:, :nw], ones_r[:],
                                         bv_t[:, n0:n0 + nw],
                                         start=False, stop=True)
                        nc.vector.tensor_copy(
                            v_sb[:, rt, h0:h0 + nw // 64, 0:64],
                            acc[:, :nw].rearrange("p (h d) -> p h d", d=64))
                v_in = dramp.tile([VLEN], BF16, tag="vin")
                nc.sync.dma_start(
                    v_in[:].rearrange("(o p x) -> p o x", p=128, x=H * 65),
                    v_sb[:])
                v_out = dramp.tile([2, VLEN], BF16, tag="vout")
                if sim:
                    for rank in range(2):
                        nc.sync.dma_start(v_out[rank], v_in[:])
                else:
                    nc.gpsimd.collective_compute(
                        "AllGather", OP.bypass,
                        replica_groups=[[0, 1], [2, 3], [4, 5], [6, 7]],
                        ins=[v_in[:].opt()], outs=[v_out[:].opt()])

                # --- qT = (h Wq)^T + bq ---
                qT = attnp.tile([128, KO, R], BF16, tag="qt")
                for m in range(KO):
                    wqm = wsmall.tile([128, KO, 128], BF16, tag="wq")
                    nc.sync.dma_start(
                        wqm[:], wq_d.ap()[l, m].rearrange("p (ko j) -> p ko j", ko=KO))
                    acc = ps.tile([128, 512], F32, tag="acc")
                    for k in range(KO):
                        nc.tensor.matmul(acc[:], wqm[:, k, :], hT[:, k, :],
                                         start=(k == 0), stop=(k == KO - 1))
                    nc.vector.tensor_scalar_add(qT[:, m, :], acc[:],
                                                bqT[:, m:m + 1])

                # --- assemble gathered kT / vext in natural key order ---
                # natural chunk j came from pair-member j%2, its block j//2
                kTg = attnp.tile([128, KO, T], BF16, tag="ktg")
                vext = attnp.tile([128, 8, H, 65], BF16, tag="vext")
                for j in range(8):
                    nc.sync.dma_start(
                        kTg[:, :, j * 128:(j + 1) * 128],
                        k_out[j % 2]
                        .rearrange("(ko p r) -> p ko r", p=128, r=R)
                        [:, :, (j // 2) * 128:(j // 2) * 128 + 128])
                    nc.sync.dma_start(
                        vext[:, j, :, :],
                        v_out[j % 2]
                        .rearrange("(o p h e) -> p o h e", p=128, h=H, e=65)
                        [:, j // 2])

                # --- attention, row-group pipelined; causal suffix-N chunks.
                # Chunk j covers local query blocks p >= j//2; within group g
                # (blocks 2g, 2g+1) its query range is [n0:256], n0 =
                # 128*max(j//2-2g, 0).  Chunks processed in descending j so
                # the per-element has_written bits make the suffix
                # accumulation correct.  The top two chunks of each block are
                # masked (tril/zeros/ones per core); mask slots are stored in
                # descending-chunk order [3,2,1,0,7,6,5,4]. ---
                yT2 = attnp.tile([128, KO, R], BF16, tag="yt")

                def attn_sub(g, hp, sub):
                    """S -> exp -> mask -> AV chain for one head, one row
                    group.  Returns the [65, 256] PSUM accumulator."""
                    jmax = 4 * g + 3
                    p0 = 64 * sub
                    ya = psy.tile([128, 256], F32, tag="ya")
                    chunks = list(range(jmax, -1, -1))
                    pts = []
                    for pi in range(len(chunks) // 2):
                        ja, jb = chunks[2 * pi], chunks[2 * pi + 1]
                        n0 = 128 * max(ja // 2 - 2 * g, 0)
                        N = 256 - n0
                        s2 = ps.tile([128, 2, 256], F32, tag="acc")
                        for jj, j in enumerate((ja, jb)):
                            nc.tensor.matmul(
                                s2[:, jj, 0:N],
                                kTg[p0:p0 + 64, hp, j * 128:(j + 1) * 128],
                                qT[p0:p0 + 64, hp,
                                   256 * g + n0:256 * g + 256],
                                start=(jj == 0), stop=(jj == 1))
                        pt = ptp.tile([128, 2, 256], BF16, tag="pt")
                        nc.scalar.activation(pt[:, :, 0:N], s2[:, :, 0:N],
                                             AF.Exp, scale=SCALE)
                        if pi < 2:
                            nc.vector.tensor_mul(
                                pt[:, :, 0:128], pt[:, :, 0:128],
                                mask_sb[:, 4 * g + 2 * pi:4 * g + 2 * pi + 2,
                                        :])
                        pts.append(pt)
                    for pi in range(len(chunks) // 2):
                        for jj in range(2):
                            j = chunks[2 * pi + jj]
                            n0 = 128 * max(j // 2 - 2 * g, 0)
                            nc.tensor.matmul(
                                ya[0:65, n0:256],
                                vext[:, j, 2 * hp + sub, :],
                                pts[pi][:, jj, 0:256 - n0],
                                start=(j == jmax), stop=(j == 0))
                    return ya

                def attn_norm(g, hp, sub, ya):
                    rl = act1.tile([1, 256], F32, tag="rl")
                    nc.vector.reciprocal(rl[:], ya[64:65, :])
                    rlb = act1.tile([64, 256], F32, tag="rlb")
                    nc.gpsimd.partition_broadcast(rlb[:], rl[:])
                    qc = 256 * g
                    if sub == 0:
                        nc.vector.tensor_tensor(yT2[0:64, hp, qc:qc + 256],
                                                ya[0:64, :], rlb[:], OP.mult)
                    else:
                        yodd = act1.tile([64, 256], BF16, tag="yodd")
                        nc.vector.tensor_tensor(yodd[:], ya[0:64, :],
                                                rlb[:], OP.mult)
                        nc.sync.dma_start(yT2[64:128, hp, qc:qc + 256],
                                          yodd[:])

                for g in range(2):
                    for hp in range(H // 2):
                        yas = [attn_sub(g, hp, sub) for sub in range(2)]
                        for sub in range(2):
                            attn_norm(g, hp, sub, yas[sub])

                # --- per group: proj, LN2, MLP (g1's attention overlaps
                # g0's proj/MLP via the scheduler) ---
                def proj_group(g):
                    for nch, (n0, nw) in enumerate(((0, 512), (512, 256))):
                        for rt in (2 * g, 2 * g + 1):
                            acc = ps.tile([128, 512], F32, tag="acc")
                            for hp in range(KO):
                                nc.tensor.matmul(
                                    acc[:, :nw],
                                    yT2[:, hp, rt * 128:(rt + 1) * 128],
                                    won[:, hp, n0:n0 + nw],
                                    start=(hp == 0), stop=False)
                            nc.tensor.matmul(acc[:, :nw], ones_r[:],
                                             bo_t[:, n0:n0 + nw],
                                             start=False, stop=True)
                            nc.vector.tensor_tensor(x_sb[:, rt, n0:n0 + nw],
                                                    x_sb[:, rt, n0:n0 + nw],
                                                    acc[:, :nw], OP.add)

                def mlp_group(g, hT2):
                    gts = []
                    for m in range(FF // 128):
                        w1m = wsmall.tile([128, KO, 128], BF16, tag="w1")
                        nc.sync.dma_start(
                            w1m[:],
                            w1_d.ap()[l, m].rearrange("p (ko j) -> p ko j",
                                                      ko=KO))
                        gacc = ps.tile([128, 512], F32, tag="acc")
                        for k in range(KO):
                            nc.tensor.matmul(
                                gacc[:, :256], w1m[:, k, :], hT2[:, k, :],
                                start=(k == 0), stop=(k == KO - 1))
                        gt_ = gtp.tile([128, 256], BF16, tag=f"g{m}")
                        nc.scalar.activation(gt_[:], gacc[:, :256], AF.Gelu,
                                             bias=b1T[:, m:m + 1])
                        gts.append(gt_)
                    for ri in range(2):
                        rt = 2 * g + ri
                        a = psf.tile([128, 512], F32, tag="fa")
                        b = psf.tile([128, 256], F32, tag="fb")
                        for m in range(FF // 128):
                            gsl = gts[m][:, ri * 128:(ri + 1) * 128]
                            nc.tensor.matmul(a[:], gsl, w2c[:, m, 0:512],
                                             start=(m == 0), stop=False)
                            nc.tensor.matmul(b[:], gsl, w2c[:, m, 512:768],
                                             start=(m == 0), stop=False)
                        for acc, n0, nw in ((a, 0, 512), (b, 512, 256)):
                            nc.tensor.matmul(acc[:, :nw], ones_r[:],
                                             b2_t[:, n0:n0 + nw],
                                             start=False, stop=True)
                            nc.vector.tensor_tensor(x_sb[:, rt, n0:n0 + nw],
                                                    x_sb[:, rt, n0:n0 + nw],
                                                    acc[:, :nw], OP.add)

                for g in range(2):
                    proj_group(g)
                    hT2 = layernorm_T(g2T, h2T, (2 * g, 2 * g + 1),
                                      f"ht2_{g}")
                    mlp_group(g, hT2)

            # ---- final LN + head ----
            gfT = cvec.tile([128, KO], F32, tag="g1T")
            nc.sync.dma_start(gfT[:], gf_d.ap().rearrange("(o p) -> p o", p=128))
            hfT = cvec.tile([128, KO], F32, tag="h1T")
            nc.sync.dma_start(hfT[:], hf_d.ap().rearrange("(o p) -> p o", p=128))
            xfT = layernorm_T(gfT, hfT, (0, 1, 2, 3), "ht")
            out_r = out_d.ap().rearrange("(o p) v -> p o v", p=128)
            for vs in range(NVS):
                hwv = wmed.tile([128, KO, 512], BF16, tag="wv")
                nc.sync.dma_start(hwv[:],
                                  hw_d.ap()[:, :, vs * 512:(vs + 1) * 512])
                vw = min(512, V - vs * 512)
                for rt in range(RT):
                    acc = ps.tile([128, 512], F32, tag="acc")
                    for k in range(KO):
                        nc.tensor.matmul(acc[:],
                                         xfT[:, k, rt * 128:(rt + 1) * 128],
                                         hwv[:, k, :],
                                         start=(k == 0), stop=(k == KO - 1))
                    st = stp.tile([128, 512], BF16, tag="lo")
                    nc.vector.tensor_copy(st[:], acc[:])
                    nc.sync.dma_start(
                        out_r[:, rt, vs * 512:vs * 512 + vw], st[:, :vw])

    nc.compile()
    return nc


def _prep_inputs(inputs):
    f = lambda k: np.asarray(inputs[k], dtype=np.float32)
    bf = lambda k: np.ascontiguousarray(
        np.asarray(inputs[k], dtype=np.float32)).astype(ml_dtypes.bfloat16)

    idx = np.asarray(inputs["idx"])
    tok = f("tok_emb")
    pos = f("pos_emb")[0]                      # [T, C]
    x0 = tok[idx] + pos[None, :, :]            # [B, T, C] f32

    hw = np.zeros((128, KO, VP), dtype=ml_dtypes.bfloat16)
    hw[:, :, :V] = bf("head_w").reshape(KO, 128, V).transpose(1, 0, 2)

    def pack_kT(w):            # [L, C, C] -> [L, KO(m), 128(p), (ko j)]
        a = w.reshape(L, KO, 128, KO, 128)         # (l, ko, p, m, j)
        return np.ascontiguousarray(a.transpose(0, 3, 2, 1, 4)).reshape(
            L, KO, 128, C)

    def pack_rhs(w, p):        # [L, K, N] -> [L, p, K//p(ko), N]
        ko = w.shape[1] // p
        a = w.reshape(L, ko, p, w.shape[2])
        return np.ascontiguousarray(a.transpose(0, 2, 1, 3))

    def pack_w1(w):            # [L, C, FF] -> [L, FF//128(m), 128(p), (ko j)]
        a = w.reshape(L, KO, 128, FF // 128, 128)  # (l, ko, p, m, j)
        return np.ascontiguousarray(a.transpose(0, 3, 2, 1, 4)).reshape(
            L, FF // 128, 128, C)

    shared = {
        "wq": pack_kT(bf("wq")), "wk": pack_kT(bf("wk")),
        "wv": pack_rhs(bf("wv"), 128), "wo": pack_rhs(bf("wo"), 128),
        "w1": pack_w1(bf("w1")), "w2": bf("w2"),
        "bq": f("bq"), "bk": f("bk"), "bv": bf("bv"), "bo": bf("bo"),
        "b1": f("b1"), "b2": bf("b2"),
        "ln1g": f("ln1_g"), "ln1b": f("ln1_b"),
        "ln2g": f("ln2_g"), "ln2b": f("ln2_b"),
        "lnfg": f("lnf_g"), "lnfb": f("lnf_b"),
        "headw": hw,
    }

    tril = (np.arange(128)[:, None] <= np.arange(128)[None, :])
    slots = [3, 2, 1, 0, 7, 6, 5, 4]           # descending-chunk order
    in_maps = []
    for core in range(NC_):
        b, s = core // 2, core % 2
        rows = np.concatenate(
            [np.arange((2 * p + s) * 128, (2 * p + s) * 128 + 128)
             for p in range(RT)])
        m = np.zeros((8, 128, 128), dtype=ml_dtypes.bfloat16)
        for si, j in enumerate(slots):
            if j % 2 == 0:
                m[si] = tril if s == 0 else 1.0
            else:
                m[si] = 0.0 if s == 0 else tril
        in_maps.append(dict(
            shared,
            x0=np.ascontiguousarray(x0[b, rows]),
            maskt=m,
        ))
    return in_maps


def kernel(**inputs):
    if "nc" not in _BUILD_CACHE:
        _BUILD_CACHE["nc"] = _build_nc()
    nc = _BUILD_CACHE["nc"]

    in_maps = _prep_inputs(inputs)
    res = bass_utils.run_bass_kernel_spmd(
        nc, in_maps, core_ids=list(range(NC_)))

    out = np.empty((B, T, V), dtype=np.float32)
    for core in range(NC_):
        b, s = core // 2, core % 2
        logits = res.results[core]["logits"].astype(np.float32)
        for p in range(RT):
            out[b, (2 * p + s) * 128:(2 * p + s + 1) * 128] = \
                logits[p * 128:(p + 1) * 128]
    return out


# revision 16
# speedup vs baseline: 1.9138x; 1.9138x over previous
"""GPT-2 (L=12, B=4, T=1024, C=768, H=12, V=50257) forward pass on 8 TRN2
NeuronCores.

Sharding: 8 cores = 4 sequences x 2 cores.  Core (b, s) owns the
INTERLEAVED query blocks {s, s+2, s+4, s+6} (128 rows each) of sequence b,
so both cores of a pair need the same causal chunk profile (2,4,6,8) per
local block and 37.5% of the S/AV/exp work is statically skipped.
Per-layer pairwise AllGather of (K^T, V-ext); gathered keys are assembled
in natural order.  Causal structure: key chunk j covers only the query
blocks that can see it (suffix of the local blocks); the two top chunks of
each block get a multiplicative [128x128] mask (tril / zeros / ones, a
per-core input).

Row groups g0 = blocks {0,1}, g1 = {2,3} are pipelined: attention of g1
(ACT-engine heavy) overlaps proj/LN2/MLP of g0 (PE heavy).

Compute dtype: bf16 matmuls with fp32 PSUM accumulation; fp32 residual
stream, layernorm stats, and softmax accumulation; bf16 logits.
"""

import numpy as np
import ml_dtypes

import concourse.bass as bass
import concourse.tile as tile
import concourse.mybir as mybir
from concourse import bacc, bass_utils
from concourse.masks import make_identity

F32 = mybir.dt.float32
BF16 = mybir.dt.bfloat16
AF = mybir.ActivationFunctionType
OP = mybir.AluOpType

L, B, T, C, H, V = 12, 4, 1024, 768, 12, 50257
D = C // H            # 64
FF = 4 * C            # 3072
R = 512               # rows per core
NC_ = 8               # cores
KO = C // 128         # 6
RT = R // 128         # 4 row tiles (local blocks)
NVS = 99              # vocab slices of 512
VP = NVS * 512        # 50688 padded vocab
SCALE = 1.0 / float(np.sqrt(D))
EPS = 1e-5

_BUILD_CACHE = {}


def _build_nc(sim=False, nocoll=False, nohead=False, nlayers=L):
    nc = bacc.Bacc("TRN2", target_bir_lowering=False, debug=False,
                   num_devices=NC_)

    # ---- I/O ----
    x0_d = nc.dram_tensor("x0", [R, C], F32, kind="ExternalInput")
    mask_d = nc.dram_tensor("maskt", [8, 128, 128], BF16, kind="ExternalInput")
    wq_d = nc.dram_tensor("wq", [L, KO, 128, C], BF16, kind="ExternalInput")
    wk_d = nc.dram_tensor("wk", [L, KO, 128, C], BF16, kind="ExternalInput")
    wv_d = nc.dram_tensor("wv", [L, 128, KO, C], BF16, kind="ExternalInput")
    wo_d = nc.dram_tensor("wo", [L, 128, KO, C], BF16, kind="ExternalInput")
    w1_d = nc.dram_tensor("w1", [L, FF // 128, 128, C], BF16, kind="ExternalInput")
    w2_d = nc.dram_tensor("w2", [L, FF, C], BF16, kind="ExternalInput")
    bq_d = nc.dram_tensor("bq", [L, C], F32, kind="ExternalInput")
    bk_d = nc.dram_tensor("bk", [L, C], F32, kind="ExternalInput")
    bv_d = nc.dram_tensor("bv", [L, C], BF16, kind="ExternalInput")
    bo_d = nc.dram_tensor("bo", [L, C], BF16, kind="ExternalInput")
    b1_d = nc.dram_tensor("b1", [L, FF], F32, kind="ExternalInput")
    b2_d = nc.dram_tensor("b2", [L, C], BF16, kind="ExternalInput")
    g1_d = nc.dram_tensor("ln1g", [L, C], F32, kind="ExternalInput")
    h1_d = nc.dram_tensor("ln1b", [L, C], F32, kind="ExternalInput")
    g2_d = nc.dram_tensor("ln2g", [L, C], F32, kind="ExternalInput")
    h2_d = nc.dram_tensor("ln2b", [L, C], F32, kind="ExternalInput")
    gf_d = nc.dram_tensor("lnfg", [C], F32, kind="ExternalInput")
    hf_d = nc.dram_tensor("lnfb", [C], F32, kind="ExternalInput")
    hw_d = nc.dram_tensor("headw", [128, KO, VP], BF16, kind="ExternalInput")
    out_d = nc.dram_tensor("logits", [R, V], BF16, kind="ExternalOutput")

    KVLEN = C * R
    VLEN = RT * H * 65 * 128
    with tile.TileContext(nc) as tc:
        with (
            tc.tile_pool(name="const", bufs=1) as const,
            tc.tile_pool(name="persist", bufs=1) as persist,
            tc.tile_pool(name="wsmall", bufs=3) as wsmall,
            tc.tile_pool(name="wmed", bufs=2) as wmed,
            tc.tile_pool(name="wop", bufs=1) as wop,
            tc.tile_pool(name="wc", bufs=1) as wcp,
            tc.tile_pool(name="cvec", bufs=2) as cvec,
            tc.tile_pool(name="act1", bufs=2) as act1,
            tc.tile_pool(name="attn", bufs=1) as attnp,
            tc.tile_pool(name="pt", bufs=6) as ptp,
            tc.tile_pool(name="gt", bufs=1) as gtp,
            tc.tile_pool(name="st", bufs=3) as stp,
            tc.tile_pool(name="ps", bufs=3, space="PSUM") as ps,
            tc.tile_pool(name="psy", bufs=3, space="PSUM") as psy,
            tc.tile_pool(name="psf", bufs=1, space="PSUM") as psf,
            tc.tile_pool(name="dram", bufs=2, space="DRAM") as dramp,
        ):
            ident = const.tile([128, 128], BF16)
            make_identity(nc, ident[:])
            eps_t = const.tile([128, 1], F32)
            nc.vector.memset(eps_t[:], EPS)
            ones_r = const.tile([1, 128], BF16)
            nc.vector.memset(ones_r[:], 1.0)
            mask_sb = const.tile([128, 8, 128], BF16)
            nc.sync.dma_start(mask_sb[:], mask_d.ap().rearrange("j k q -> k j q"))

            # residual stream, fp32, [128, rt, C]; rt = local block
            x_sb = persist.tile([128, RT, C], F32)
            nc.sync.dma_start(
                x_sb[:], x0_d.ap().rearrange("(o p) c -> p o c", p=128))

            def layernorm_T(gT, bT, rts, tag):
                """LN(x[rts]) -> bf16, transposed into hT [128, KO, 128*len(rts)].

                gT/bT are [128, KO] partition-form gain/bias, applied after
                the transpose (feature dim lands on partitions there)."""
                hT = attnp.tile([128, KO, 128 * len(rts)], BF16, tag=tag)
                for ri, rt in enumerate(rts):
                    xs = x_sb[:, rt, :]
                    stats = act1.tile([128, 3, 6], F32, tag=f"st{tag}")
                    xs3 = xs.rearrange("p (s d) -> p s d", s=3)
                    for s in range(3):
                        nc.vector.bn_stats(stats[:, s, :], xs3[:, s, :])
                    mv = act1.tile([128, 2], F32, tag=f"mv{tag}")
                    nc.vector.bn_aggr(mv[:], stats[:])
                    std = act1.tile([128, 1], F32, tag=f"sd{tag}")
                    nc.scalar.activation(std[:], mv[:, 1:2], AF.Sqrt,
                                         bias=eps_t[:])
                    nc.vector.reciprocal(std[:], std[:])
                    hnb = act1.tile([128, C], BF16, tag=f"hb{tag}")
                    nc.vector.tensor_scalar(hnb[:], xs, mv[:, 0:1], std[:],
                                            op0=OP.subtract, op1=OP.mult)
                    for ko in range(KO):
                        pt_ = ps.tile([128, 512], BF16, tag="acc")
                        nc.tensor.transpose(pt_[:, :128],
                                            hnb[:, ko * 128:(ko + 1) * 128],
                                            ident[:])
                        nc.vector.tensor_scalar(
                            hT[:, ko, ri * 128:(ri + 1) * 128], pt_[:, :128],
                            gT[:, ko:ko + 1], bT[:, ko:ko + 1],
                            op0=OP.mult, op1=OP.add)
                return hT

            for l in range(nlayers):
                # --- per-layer constant vectors ---
                bqT = cvec.tile([128, KO], F32, tag="bqT")
                nc.sync.dma_start(bqT[:], bq_d.ap()[l].rearrange("(o p) -> p o", p=128))
                bkT = cvec.tile([128, KO], F32, tag="bkT")
                nc.sync.dma_start(bkT[:], bk_d.ap()[l].rearrange("(o p) -> p o", p=128))
                b1T = cvec.tile([128, FF // 128], F32, tag="b1T")
                nc.sync.dma_start(b1T[:], b1_d.ap()[l].rearrange("(o p) -> p o", p=128))
                g1T = cvec.tile([128, KO], F32, tag="g1T")
                nc.sync.dma_start(g1T[:], g1_d.ap()[l].rearrange("(o p) -> p o", p=128))
                h1T = cvec.tile([128, KO], F32, tag="h1T")
                nc.sync.dma_start(h1T[:], h1_d.ap()[l].rearrange("(o p) -> p o", p=128))
                g2T = cvec.tile([128, KO], F32, tag="g2T")
                nc.sync.dma_start(g2T[:], g2_d.ap()[l].rearrange("(o p) -> p o", p=128))
                h2T = cvec.tile([128, KO], F32, tag="h2T")
                nc.sync.dma_start(h2T[:], h2_d.ap()[l].rearrange("(o p) -> p o", p=128))
                bv_t = cvec.tile([1, C], BF16, tag="bv")
                nc.sync.dma_start(bv_t[:], bv_d.ap()[l][None, :])
                bo_t = cvec.tile([1, C], BF16, tag="bo")
                nc.sync.dma_start(bo_t[:], bo_d.ap()[l][None, :])
                b2_t = cvec.tile([1, C], BF16, tag="b2")
                nc.sync.dma_start(b2_t[:], b2_d.ap()[l][None, :])
                # prefetch the full-layer wo / w2 caches early
                won = wop.tile([128, KO, C], BF16, tag="wo")
                nc.sync.dma_start(won[:], wo_d.ap()[l])
                w2c = wcp.tile([128, FF // 128, C], BF16, tag="w2c")
                nc.sync.dma_start(
                    w2c[:], w2_d.ap()[l].rearrange("(m p) c -> p m c", p=128))

                # --- LN1 -> hT ---
                hT = layernorm_T(g1T, h1T, (0, 1, 2, 3), "ht")

                # --- kT = (h Wk)^T + bk ---  [128, KO, R]
                kT = attnp.tile([128, KO, R], BF16, tag="kt")
                for m in range(KO):
                    wkm = wsmall.tile([128, KO, 128], BF16, tag="wk")
                    nc.sync.dma_start(
                        wkm[:], wk_d.ap()[l, m].rearrange("p (ko j) -> p ko j", ko=KO))
                    acc = ps.tile([128, 512], F32, tag="acc")
                    for k in range(KO):
                        nc.tensor.matmul(acc[:], wkm[:, k, :], hT[:, k, :],
                                         start=(k == 0), stop=(k == KO - 1))
                    nc.vector.tensor_scalar_add(kT[:, m, :], acc[:],
                                                bkT[:, m:m + 1])
                k_in = dramp.tile([KVLEN], BF16, tag="kin")
                nc.sync.dma_start(
                    k_in[:].rearrange("(ko p r) -> p ko r", p=128, r=R),
                    kT[:])
                k_out = dramp.tile([2, KVLEN], BF16, tag="kout")
                if nocoll:
                    pass
                elif sim:
                    for rank in range(2):
                        nc.sync.dma_start(k_out[rank], k_in[:])
                else:
                    nc.gpsimd.collective_compute(
                        "AllGather", OP.bypass,
                        replica_groups=[[0, 1], [2, 3], [4, 5], [6, 7]],
                        ins=[k_in[:].opt()], outs=[k_out[:].opt()])

                # --- v = h Wv + bv ---  stored vext-shaped [128, RT, H, 65]
                v_sb = attnp.tile([128, RT, H, 65], BF16, tag="v")
                nc.vector.memset(v_sb[:, :, :, 64:65], 1.0)
                for nch, (n0, nw) in enumerate(((0, 512), (512, 256))):
                    wvn = wmed.tile([128, KO, 512], BF16, tag="wv")
                    nc.sync.dma_start(wvn[:, :, :nw],
                                      wv_d.ap()[l][:, :, n0:n0 + nw])
                    h0 = n0 // 64
                    for rt in range(RT):
                        acc = ps.tile([128, 512], F32, tag="acc")
                        for k in range(KO):
                            nc.tensor.matmul(
                                acc[:, :nw], hT[:, k, rt * 128:(rt + 1) * 128],
                                wvn[:, k, :nw],
                                start=(k == 0), stop=False)
                        nc.tensor.matmul(acc[:, :nw], ones_r[:],
                                         bv_t[:, n0:n0 + nw],
                                         start=False, stop=True)
                        nc.vector.tensor_copy(
                            v_sb[:, rt, h0:h0 + nw // 64, 0:64],
                            acc[:, :nw].rearrange("p (h d) -> p h d", d=64))
                v_in = dramp.tile([VLEN], BF16, tag="vin")
                nc.sync.dma_start(
                    v_in[:].rearrange("(o p x) -> p o x", p=128, x=H * 65),
                    v_sb[:])
                v_out = dramp.tile([2, VLEN], BF16, tag="vout")
                if nocoll:
                    pass
                elif sim:
                    for rank in range(2):
                        nc.sync.dma_start(v_out[rank], v_in[:])
                else:
                    nc.gpsimd.collective_compute(
                        "AllGather", OP.bypass,
                        replica_groups=[[0, 1], [2, 3], [4, 5], [6, 7]],
                        ins=[v_in[:].opt()], outs=[v_out[:].opt()])

                # --- qT = (h Wq)^T + bq ---
                qT = attnp.tile([128, KO, R], BF16, tag="qt")
                for m in range(KO):
                    wqm = wsmall.tile([128, KO, 128], BF16, tag="wq")
                    nc.sync.dma_start(
                        wqm[:], wq_d.ap()[l, m].rearrange("p (ko j) -> p ko j", ko=KO))
                    acc = ps.tile([128, 512], F32, tag="acc")
                    for k in range(KO):
                        nc.tensor.matmul(acc[:], wqm[:, k, :], hT[:, k, :],
                                         start=(k == 0), stop=(k == KO - 1))
                    nc.vector.tensor_scalar_add(qT[:, m, :], acc[:],
                                                bqT[:, m:m + 1])

                # --- assemble gathered kT / vext in natural key order ---
                # natural chunk j came from pair-member j%2, its block j//2
                kTg = attnp.tile([128, KO, T], BF16, tag="ktg")
                vext = attnp.tile([128, 8, H, 65], BF16, tag="vext")
                for j in range(8):
                    k_src = (k_in[:] if nocoll else k_out[j % 2])
                    v_src = (v_in[:] if nocoll else v_out[j % 2])
                    nc.sync.dma_start(
                        kTg[:, :, j * 128:(j + 1) * 128],
                        k_src
                        .rearrange("(ko p r) -> p ko r", p=128, r=R)
                        [:, :, (j // 2) * 128:(j // 2) * 128 + 128])
                    nc.sync.dma_start(
                        vext[:, j, :, :],
                        v_src
                        .rearrange("(o p h e) -> p o h e", p=128, h=H, e=65)
                        [:, j // 2])

                # --- attention, row-group pipelined; causal suffix-N chunks.
                # Chunk j covers local query blocks p >= j//2; within group g
                # (blocks 2g, 2g+1) its query range is [n0:256], n0 =
                # 128*max(j//2-2g, 0).  Chunks processed in descending j so
                # the per-element has_written bits make the suffix
                # accumulation correct.  The top two chunks of each block are
                # masked (tril/zeros/ones per core); mask slots are stored in
                # descending-chunk order [3,2,1,0,7,6,5,4]. ---
                yT2 = attnp.tile([128, KO, R], BF16, tag="yt")

                def attn_sub(g, hp, sub):
                    """S -> exp -> mask -> AV chain for one head, one row
                    group.  Returns the [65, 256] PSUM accumulator."""
                    jmax = 4 * g + 3
                    p0 = 64 * sub
                    ya = psy.tile([128, 256], F32, tag="ya")
                    chunks = list(range(jmax, -1, -1))
                    pts = []
                    for pi in range(len(chunks) // 2):
                        ja, jb = chunks[2 * pi], chunks[2 * pi + 1]
                        n0 = 128 * max(ja // 2 - 2 * g, 0)
                        N = 256 - n0
                        s2 = ps.tile([128, 2, 256], F32, tag="acc")
                        for jj, j in enumerate((ja, jb)):
                            nc.tensor.matmul(
                                s2[:, jj, 0:N],
                                kTg[p0:p0 + 64, hp, j * 128:(j + 1) * 128],
                                qT[p0:p0 + 64, hp,
                                   256 * g + n0:256 * g + 256],
                                start=(jj == 0), stop=(jj == 1))
                        pt = ptp.tile([128, 2, 256], BF16, tag="pt")
                        nc.scalar.activation(pt[:, :, 0:N], s2[:, :, 0:N],
                                             AF.Exp, scale=SCALE)
                        if pi < 2:
                            nc.vector.tensor_mul(
                                pt[:, :, 0:128], pt[:, :, 0:128],
                                mask_sb[:, 4 * g + 2 * pi:4 * g + 2 * pi + 2,
                                        :])
                        pts.append(pt)
                    for pi in range(len(chunks) // 2):
                        for jj in range(2):
                            j = chunks[2 * pi + jj]
                            n0 = 128 * max(j // 2 - 2 * g, 0)
                            nc.tensor.matmul(
                                ya[0:65, n0:256],
                                vext[:, j, 2 * hp + sub, :],
                                pts[pi][:, jj, 0:256 - n0],
                                start=(j == jmax), stop=(j == 0))
                    return ya

                def attn_norm(g, hp, sub, ya):
                    rl = act1.tile([1, 256], F32, tag="rl")
                    nc.vector.reciprocal(rl[:], ya[64:65, :])
                    rlb = act1.tile([64, 256], F32, tag="rlb")
                    nc.gpsimd.partition_broadcast(rlb[:], rl[:])
                    qc = 256 * g
                    if sub == 0:
                        nc.vector.tensor_tensor(yT2[0:64, hp, qc:qc + 256],
                                                ya[0:64, :], rlb[:], OP.mult)
                    else:
                        yodd = act1.tile([64, 256], BF16, tag="yodd")
                        nc.vector.tensor_tensor(yodd[:], ya[0:64, :],
                                                rlb[:], OP.mult)
                        nc.sync.dma_start(yT2[64:128, hp, qc:qc + 256],
                                          yodd[:])

                for g in range(2):
                    for hp in range(H // 2):
                        yas = [attn_sub(g, hp, sub) for sub in range(2)]
                        for sub in range(2):
                            attn_norm(g, hp, sub, yas[sub])

                # --- per group: proj, LN2, MLP (g1's attention overlaps
                # g0's proj/MLP via the scheduler) ---
                def proj_group(g):
                    for nch, (n0, nw) in enumerate(((0, 512), (512, 256))):
                        for rt in (2 * g, 2 * g + 1):
                            acc = ps.tile([128, 512], F32, tag="acc")
                            for hp in range(KO):
                                nc.tensor.matmul(
                                    acc[:, :nw],
                                    yT2[:, hp, rt * 128:(rt + 1) * 128],
                                    won[:, hp, n0:n0 + nw],
                                    start=(hp == 0), stop=False)
                            nc.tensor.matmul(acc[:, :nw], ones_r[:],
                                             bo_t[:, n0:n0 + nw],
                                             start=False, stop=True)
                            nc.vector.tensor_tensor(x_sb[:, rt, n0:n0 + nw],
                                                    x_sb[:, rt, n0:n0 + nw],
                                                    acc[:, :nw], OP.add)

                def mlp_group(g, hT2):
                    gts = []
                    for m in range(FF // 128):
                        w1m = wsmall.tile([128, KO, 128], BF16, tag="w1")
                        nc.sync.dma_start(
                            w1m[:],
                            w1_d.ap()[l, m].rearrange("p (ko j) -> p ko j",
                                                      ko=KO))
                        gacc = ps.tile([128, 512], F32, tag="acc")
                        for k in range(KO):
                            nc.tensor.matmul(
                                gacc[:, :256], w1m[:, k, :], hT2[:, k, :],
                                start=(k == 0), stop=(k == KO - 1))
                        gt_ = gtp.tile([128, 256], BF16, tag=f"g{m}")
                        nc.scalar.activation(gt_[:], gacc[:, :256], AF.Gelu,
                                             bias=b1T[:, m:m + 1])
                        gts.append(gt_)
                    for ri in range(2):
                        rt = 2 * g + ri
                        a = psf.tile([128, 512], F32, tag="fa")
                        b = psf.tile([128, 256], F32, tag="fb")
                        for m in range(FF // 128):
                            gsl = gts[m][:, ri * 128:(ri + 1) * 128]
                            nc.tensor.matmul(a[:], gsl, w2c[:, m, 0:512],
                                             start=(m == 0), stop=False)
                            nc.tensor.matmul(b[:], gsl, w2c[:, m, 512:768],
                                             start=(m == 0), stop=False)
                        for acc, n0, nw in ((a, 0, 512), (b, 512, 256)):
                            nc.tensor.matmul(acc[:, :nw], ones_r[:],
                                             b2_t[:, n0:n0 + nw],
                                             start=False, stop=True)
                            nc.vector.tensor_tensor(x_sb[:, rt, n0:n0 + nw],
                                                    x_sb[:, rt, n0:n0 + nw],
                                                    acc[:, :nw], OP.add)

                for g in range(2):
                    proj_group(g)
                    hT2 = layernorm_T(g2T, h2T, (2 * g, 2 * g + 1),
                                      f"ht2_{g}")
                    mlp_group(g, hT2)

            # ---- final LN + head ----
            gfT = cvec.tile([128, KO], F32, tag="g1T")
            nc.sync.dma_start(gfT[:], gf_d.ap().rearrange("(o p) -> p o", p=128))
            hfT = cvec.tile([128, KO], F32, tag="h1T")
            nc.sync.dma_start(hfT[:], hf_d.ap().rearrange("(o p) -> p o", p=128))
            xfT = layernorm_T(gfT, hfT, (0, 1, 2, 3), "ht")
            out_r = out_d.ap().rearrange("(o p) v -> p o v", p=128)
            for vs in range(1 if nohead else NVS):
                hwv = wmed.tile([128, KO, 512], BF16, tag="wv")
                nc.sync.dma_start(hwv[:],
                                  hw_d.ap()[:, :, vs * 512:(vs + 1) * 512])
                vw = min(512, V - vs * 512)
                for rt in range(RT):
                    acc = ps.tile([128, 512], F32, tag="acc")
                    for k in range(KO):
                        nc.tensor.matmul(acc[:],
                                         xfT[:, k, rt * 128:(rt + 1) * 128],
                                         hwv[:, k, :],
                                         start=(k == 0), stop=(k == KO - 1))
                    st = stp.tile([128, 512], BF16, tag="lo")
                    nc.vector.tensor_copy(st[:], acc[:])
                    nc.sync.dma_start(
                        out_r[:, rt, vs * 512:vs * 512 + vw], st[:, :vw])

    nc.compile()
    return nc


def _prep_inputs(inputs):
    f = lambda k: np.asarray(inputs[k], dtype=np.float32)
    bf = lambda k: np.ascontiguousarray(
        np.asarray(inputs[k], dtype=np.float32)).astype(ml_dtypes.bfloat16)

    idx = np.asarray(inputs["idx"])
    tok = f("tok_emb")
    pos = f("pos_emb")[0]                      # [T, C]
    x0 = tok[idx] + pos[None, :, :]            # [B, T, C] f32

    hw = np.zeros((128, KO, VP), dtype=ml_dtypes.bfloat16)
    hw[:, :, :V] = bf("head_w").reshape(KO, 128, V).transpose(1, 0, 2)

    def pack_kT(w):            # [L, C, C] -> [L, KO(m), 128(p), (ko j)]
        a = w.reshape(L, KO, 128, KO, 128)         # (l, ko, p, m, j)
        return np.ascontiguousarray(a.transpose(0, 3, 2, 1, 4)).reshape(
            L, KO, 128, C)

    def pack_rhs(w, p):        # [L, K, N] -> [L, p, K//p(ko), N]
        ko = w.shape[1] // p
        a = w.reshape(L, ko, p, w.shape[2])
        return np.ascontiguousarray(a.transpose(0, 2, 1, 3))

    def pack_w1(w):            # [L, C, FF] -> [L, FF//128(m), 128(p), (ko j)]
        a = w.reshape(L, KO, 128, FF // 128, 128)  # (l, ko, p, m, j)
        return np.ascontiguousarray(a.transpose(0, 3, 2, 1, 4)).reshape(
            L, FF // 128, 128, C)

    shared = {
        "wq": pack_kT(bf("wq")), "wk": pack_kT(bf("wk")),
        "wv": pack_rhs(bf("wv"), 128), "wo": pack_rhs(bf("wo"), 128),
        "w1": pack_w1(bf("w1")), "w2": bf("w2"),
        "bq": f("bq"), "bk": f("bk"), "bv": bf("bv"), "bo": bf("bo"),
        "b1": f("b1"), "b2": bf("b2"),
        "ln1g": f("ln1_g"), "ln1b": f("ln1_b"),
        "ln2g": f("ln2_g"), "ln2b": f("ln2_b"),
        "lnfg": f("lnf_g"), "lnfb": f("lnf_b"),
        "headw": hw,
    }

    tril = (np.arange(128)[:, None] <= np.arange(128)[None, :])
    slots = [3, 2, 1, 0, 7, 6, 5, 4]           # descending-chunk order
    in_maps = []
    for core in range(NC_):
        b, s = core // 2, core % 2
        rows = np.concatenate(
            [np.arange((2 * p + s) * 128, (2 * p + s) * 128 + 128)
             for p in range(RT)])
        m = np.zeros((8, 128, 128), dtype=ml_dtypes.bfloat16)
        for si, j in enumerate(slots):
            if j % 2 == 0:
                m[si] = tril if s == 0 else 1.0
            else:
                m[si] = 0.0 if s == 0 else tril
        in_maps.append(dict(
            shared,
            x0=np.ascontiguousarray(x0[b, rows]),
            maskt=m,
        ))
    return in_maps


def kernel(**inputs):
    if "nc" not in _BUILD_CACHE:
        _BUILD_CACHE["nc"] = _build_nc()
    nc = _BUILD_CACHE["nc"]

    in_maps = _prep_inputs(inputs)
    res = bass_utils.run_bass_kernel_spmd(
        nc, in_maps, core_ids=list(range(NC_)))

    out = np.empty((B, T, V), dtype=np.float32)
    for core in range(NC_):
        b, s = core // 2, core % 2
        logits = res.results[core]["logits"].astype(np.float32)
        for p in range(RT):
            out[b, (2 * p + s) * 128:(2 * p + s + 1) * 128] = \
                logits[p * 128:(p + 1) * 128]
    return out


# revision 17
# speedup vs baseline: 6.4204x; 3.3548x over previous
"""GPT-2 (L=12, B=4, T=1024, C=768, H=12, V=50257) forward pass on 8 TRN2
NeuronCores.

Sharding: 8 cores = 4 sequences x 2 cores.  Core (b, s) owns the
INTERLEAVED query blocks {s, s+2, s+4, s+6} (128 rows each) of sequence b,
so both cores of a pair need the same causal chunk profile (2,4,6,8) per
local block and 37.5% of the S/AV/exp work is statically skipped.
Per-layer pairwise AllGather of (K^T, V-ext); gathered keys are assembled
in natural order.  Causal structure: key chunk j covers only the query
blocks that can see it (suffix of the local blocks); the two top chunks of
each block get a multiplicative [128x128] mask (tril / zeros / ones, a
per-core input).

Row groups g0 = blocks {0,1}, g1 = {2,3} are pipelined: attention of g1
(ACT-engine heavy) overlaps proj/LN2/MLP of g0 (PE heavy).

Compute dtype: bf16 matmuls with fp32 PSUM accumulation; fp32 residual
stream, layernorm stats, and softmax accumulation; bf16 logits.
"""

import numpy as np
import ml_dtypes

import concourse.bass as bass
import concourse.tile as tile
import concourse.mybir as mybir
from concourse import bacc, bass_utils
from concourse.masks import make_identity

F32 = mybir.dt.float32
BF16 = mybir.dt.bfloat16
AF = mybir.ActivationFunctionType
OP = mybir.AluOpType

L, B, T, C, H, V = 12, 4, 1024, 768, 12, 50257
D = C // H            # 64
FF = 4 * C            # 3072
R = 512               # rows per core
NC_ = 8               # cores
KO = C // 128         # 6
RT = R // 128         # 4 row tiles (local blocks)
NVS = 99              # vocab slices of 512
VP = NVS * 512        # 50688 padded vocab
SCALE = 1.0 / float(np.sqrt(D))
EPS = 1e-5

_BUILD_CACHE = {}
_WMED_BUFS = 2
_HEAD_M = 128


def _build_nc(sim=False, nocoll=False, nohead=False, nlayers=L):
    nc = bacc.Bacc("TRN2", target_bir_lowering=False, debug=False,
                   num_devices=NC_)

    # ---- I/O ----
    x0_d = nc.dram_tensor("x0", [R, C], F32, kind="ExternalInput")
    mask_d = nc.dram_tensor("maskt", [8, 128, 128], BF16, kind="ExternalInput")
    wq_d = nc.dram_tensor("wq", [L, KO, 128, C], BF16, kind="ExternalInput")
    wk_d = nc.dram_tensor("wk", [L, KO, 128, C], BF16, kind="ExternalInput")
    wv_d = nc.dram_tensor("wv", [L, 128, KO, C], BF16, kind="ExternalInput")
    wo_d = nc.dram_tensor("wo", [L, 128, KO, C], BF16, kind="ExternalInput")
    w1_d = nc.dram_tensor("w1", [L, FF // 128, 128, C], BF16, kind="ExternalInput")
    w2_d = nc.dram_tensor("w2", [L, FF, C], BF16, kind="ExternalInput")
    bq_d = nc.dram_tensor("bq", [L, C], F32, kind="ExternalInput")
    bk_d = nc.dram_tensor("bk", [L, C], F32, kind="ExternalInput")
    bv_d = nc.dram_tensor("bv", [L, C], BF16, kind="ExternalInput")
    bo_d = nc.dram_tensor("bo", [L, C], BF16, kind="ExternalInput")
    b1_d = nc.dram_tensor("b1", [L, FF], F32, kind="ExternalInput")
    b2_d = nc.dram_tensor("b2", [L, C], BF16, kind="ExternalInput")
    g1_d = nc.dram_tensor("ln1g", [L, C], F32, kind="ExternalInput")
    h1_d = nc.dram_tensor("ln1b", [L, C], F32, kind="ExternalInput")
    g2_d = nc.dram_tensor("ln2g", [L, C], F32, kind="ExternalInput")
    h2_d = nc.dram_tensor("ln2b", [L, C], F32, kind="ExternalInput")
    gf_d = nc.dram_tensor("lnfg", [C], F32, kind="ExternalInput")
    hf_d = nc.dram_tensor("lnfb", [C], F32, kind="ExternalInput")
    hw_d = nc.dram_tensor("headw", [128, KO, VP], BF16, kind="ExternalInput")
    out_d = nc.dram_tensor("logits", [R, V], BF16, kind="ExternalOutput")

    KVLEN = C * R
    VLEN = RT * H * 65 * 128
    with tile.TileContext(nc) as tc:
        with (
            tc.tile_pool(name="const", bufs=1) as const,
            tc.tile_pool(name="persist", bufs=1) as persist,
            tc.tile_pool(name="wsmall", bufs=3) as wsmall,
            tc.tile_pool(name="wmed", bufs=_WMED_BUFS) as wmed,
            tc.tile_pool(name="wop", bufs=1) as wop,
            tc.tile_pool(name="wc", bufs=1) as wcp,
            tc.tile_pool(name="cvec", bufs=2) as cvec,
            tc.tile_pool(name="act1", bufs=2) as act1,
            tc.tile_pool(name="attn", bufs=1) as attnp,
            tc.tile_pool(name="pt", bufs=6) as ptp,
            tc.tile_pool(name="gt", bufs=1) as gtp,
            tc.tile_pool(name="st", bufs=3) as stp,
            tc.tile_pool(name="ps", bufs=3, space="PSUM") as ps,
            tc.tile_pool(name="psy", bufs=3, space="PSUM") as psy,
            tc.tile_pool(name="psf", bufs=1, space="PSUM") as psf,
            tc.tile_pool(name="dram", bufs=2, space="DRAM") as dramp,
        ):
            ident = const.tile([128, 128], BF16)
            make_identity(nc, ident[:])
            eps_t = const.tile([128, 1], F32)
            nc.vector.memset(eps_t[:], EPS)
            ones_r = const.tile([1, 128], BF16)
            nc.vector.memset(ones_r[:], 1.0)
            mask_sb = const.tile([128, 8, 128], BF16)
            nc.sync.dma_start(mask_sb[:], mask_d.ap().rearrange("j k q -> k j q"))

            # residual stream, fp32, [128, rt, C]; rt = local block
            x_sb = persist.tile([128, RT, C], F32)
            nc.sync.dma_start(
                x_sb[:], x0_d.ap().rearrange("(o p) c -> p o c", p=128))

            def layernorm_T(gT, bT, rts, tag):
                """LN(x[rts]) -> bf16, transposed into hT [128, KO, 128*len(rts)].

                gT/bT are [128, KO] partition-form gain/bias, applied after
                the transpose (feature dim lands on partitions there)."""
                hT = attnp.tile([128, KO, 128 * len(rts)], BF16, tag=tag)
                for ri, rt in enumerate(rts):
                    xs = x_sb[:, rt, :]
                    stats = act1.tile([128, 3, 6], F32, tag=f"st{tag}")
                    xs3 = xs.rearrange("p (s d) -> p s d", s=3)
                    for s in range(3):
                        nc.vector.bn_stats(stats[:, s, :], xs3[:, s, :])
                    mv = act1.tile([128, 2], F32, tag=f"mv{tag}")
                    nc.vector.bn_aggr(mv[:], stats[:])
                    std = act1.tile([128, 1], F32, tag=f"sd{tag}")
                    nc.scalar.activation(std[:], mv[:, 1:2], AF.Sqrt,
                                         bias=eps_t[:])
                    nc.vector.reciprocal(std[:], std[:])
                    hnb = act1.tile([128, C], BF16, tag=f"hb{tag}")
                    nc.vector.tensor_scalar(hnb[:], xs, mv[:, 0:1], std[:],
                                            op0=OP.subtract, op1=OP.mult)
                    for ko in range(KO):
                        pt_ = ps.tile([128, 512], BF16, tag="acc")
                        nc.tensor.transpose(pt_[:, :128],
                                            hnb[:, ko * 128:(ko + 1) * 128],
                                            ident[:])
                        nc.vector.tensor_scalar(
                            hT[:, ko, ri * 128:(ri + 1) * 128], pt_[:, :128],
                            gT[:, ko:ko + 1], bT[:, ko:ko + 1],
                            op0=OP.mult, op1=OP.add)
                return hT

            for l in range(nlayers):
                # --- per-layer constant vectors ---
                bqT = cvec.tile([128, KO], F32, tag="bqT")
                nc.sync.dma_start(bqT[:], bq_d.ap()[l].rearrange("(o p) -> p o", p=128))
                bkT = cvec.tile([128, KO], F32, tag="bkT")
                nc.sync.dma_start(bkT[:], bk_d.ap()[l].rearrange("(o p) -> p o", p=128))
                b1T = cvec.tile([128, FF // 128], F32, tag="b1T")
                nc.sync.dma_start(b1T[:], b1_d.ap()[l].rearrange("(o p) -> p o", p=128))
                g1T = cvec.tile([128, KO], F32, tag="g1T")
                nc.sync.dma_start(g1T[:], g1_d.ap()[l].rearrange("(o p) -> p o", p=128))
                h1T = cvec.tile([128, KO], F32, tag="h1T")
                nc.sync.dma_start(h1T[:], h1_d.ap()[l].rearrange("(o p) -> p o", p=128))
                g2T = cvec.tile([128, KO], F32, tag="g2T")
                nc.sync.dma_start(g2T[:], g2_d.ap()[l].rearrange("(o p) -> p o", p=128))
                h2T = cvec.tile([128, KO], F32, tag="h2T")
                nc.sync.dma_start(h2T[:], h2_d.ap()[l].rearrange("(o p) -> p o", p=128))
                bv_t = cvec.tile([1, C], BF16, tag="bv")
                nc.sync.dma_start(bv_t[:], bv_d.ap()[l][None, :])
                bo_t = cvec.tile([1, C], BF16, tag="bo")
                nc.sync.dma_start(bo_t[:], bo_d.ap()[l][None, :])
                b2_t = cvec.tile([1, C], BF16, tag="b2")
                nc.sync.dma_start(b2_t[:], b2_d.ap()[l][None, :])
                # prefetch the full-layer wo / w2 caches early
                won = wop.tile([128, KO, C], BF16, tag="wo")
                nc.sync.dma_start(won[:], wo_d.ap()[l])
                w2c = wcp.tile([128, FF // 128, C], BF16, tag="w2c")
                nc.sync.dma_start(
                    w2c[:], w2_d.ap()[l].rearrange("(m p) c -> p m c", p=128))

                # --- LN1 -> hT ---
                hT = layernorm_T(g1T, h1T, (0, 1, 2, 3), "ht")

                # --- kT = (h Wk)^T + bk ---  [128, KO, R]
                kT = attnp.tile([128, KO, R], BF16, tag="kt")
                for m in range(KO):
                    wkm = wsmall.tile([128, KO, 128], BF16, tag="wk")
                    nc.sync.dma_start(
                        wkm[:], wk_d.ap()[l, m].rearrange("p (ko j) -> p ko j", ko=KO))
                    acc = ps.tile([128, 512], F32, tag="acc")
                    for k in range(KO):
                        nc.tensor.matmul(acc[:], wkm[:, k, :], hT[:, k, :],
                                         start=(k == 0), stop=(k == KO - 1))
                    nc.vector.tensor_scalar_add(kT[:, m, :], acc[:],
                                                bkT[:, m:m + 1])
                k_in = dramp.tile([KVLEN], BF16, tag="kin")
                nc.sync.dma_start(
                    k_in[:].rearrange("(ko p r) -> p ko r", p=128, r=R),
                    kT[:])
                k_out = dramp.tile([2, KVLEN], BF16, tag="kout")
                if nocoll:
                    pass
                elif sim:
                    for rank in range(2):
                        nc.sync.dma_start(k_out[rank], k_in[:])
                else:
                    nc.gpsimd.collective_compute(
                        "AllGather", OP.bypass,
                        replica_groups=[[0, 1], [2, 3], [4, 5], [6, 7]],
                        ins=[k_in[:].opt()], outs=[k_out[:].opt()])

                # --- v = h Wv + bv ---  stored vext-shaped [128, RT, H, 65]
                v_sb = attnp.tile([128, RT, H, 65], BF16, tag="v")
                nc.vector.memset(v_sb[:, :, :, 64:65], 1.0)
                for nch, (n0, nw) in enumerate(((0, 512), (512, 256))):
                    wvn = wmed.tile([128, KO, 512], BF16, tag="wv")
                    nc.sync.dma_start(wvn[:, :, :nw],
                                      wv_d.ap()[l][:, :, n0:n0 + nw])
                    h0 = n0 // 64
                    for rt in range(RT):
                        acc = ps.tile([128, 512], F32, tag="acc")
                        for k in range(KO):
                            nc.tensor.matmul(
                                acc[:, :nw], hT[:, k, rt * 128:(rt + 1) * 128],
                                wvn[:, k, :nw],
                                start=(k == 0), stop=False)
                        nc.tensor.matmul(acc[:, :nw], ones_r[:],
                                         bv_t[:, n0:n0 + nw],
                                         start=False, stop=True)
                        nc.vector.tensor_copy(
                            v_sb[:, rt, h0:h0 + nw // 64, 0:64],
                            acc[:, :nw].rearrange("p (h d) -> p h d", d=64))
                v_in = dramp.tile([VLEN], BF16, tag="vin")
                nc.sync.dma_start(
                    v_in[:].rearrange("(o p x) -> p o x", p=128, x=H * 65),
                    v_sb[:])
                v_out = dramp.tile([2, VLEN], BF16, tag="vout")
                if nocoll:
                    pass
                elif sim:
                    for rank in range(2):
                        nc.sync.dma_start(v_out[rank], v_in[:])
                else:
                    nc.gpsimd.collective_compute(
                        "AllGather", OP.bypass,
                        replica_groups=[[0, 1], [2, 3], [4, 5], [6, 7]],
                        ins=[v_in[:].opt()], outs=[v_out[:].opt()])

                # --- qT = (h Wq)^T + bq ---
                qT = attnp.tile([128, KO, R], BF16, tag="qt")
                for m in range(KO):
                    wqm = wsmall.tile([128, KO, 128], BF16, tag="wq")
                    nc.sync.dma_start(
                        wqm[:], wq_d.ap()[l, m].rearrange("p (ko j) -> p ko j", ko=KO))
                    acc = ps.tile([128, 512], F32, tag="acc")
                    for k in range(KO):
                        nc.tensor.matmul(acc[:], wqm[:, k, :], hT[:, k, :],
                                         start=(k == 0), stop=(k == KO - 1))
                    nc.vector.tensor_scalar_add(qT[:, m, :], acc[:],
                                                bqT[:, m:m + 1])

                # --- assemble gathered kT / vext in natural key order ---
                # natural chunk j came from pair-member j%2, its block j//2
                kTg = attnp.tile([128, KO, T], BF16, tag="ktg")
                vext = attnp.tile([128, 8, H, 65], BF16, tag="vext")
                for j in range(8):
                    k_src = (k_in[:] if nocoll else k_out[j % 2])
                    v_src = (v_in[:] if nocoll else v_out[j % 2])
                    nc.sync.dma_start(
                        kTg[:, :, j * 128:(j + 1) * 128],
                        k_src
                        .rearrange("(ko p r) -> p ko r", p=128, r=R)
                        [:, :, (j // 2) * 128:(j // 2) * 128 + 128])
                    nc.sync.dma_start(
                        vext[:, j, :, :],
                        v_src
                        .rearrange("(o p h e) -> p o h e", p=128, h=H, e=65)
                        [:, j // 2])

                # --- attention, row-group pipelined; causal suffix-N chunks.
                # Chunk j covers local query blocks p >= j//2; within group g
                # (blocks 2g, 2g+1) its query range is [n0:256], n0 =
                # 128*max(j//2-2g, 0).  Chunks processed in descending j so
                # the per-element has_written bits make the suffix
                # accumulation correct.  The top two chunks of each block are
                # masked (tril/zeros/ones per core); mask slots are stored in
                # descending-chunk order [3,2,1,0,7,6,5,4]. ---
                yT2 = attnp.tile([128, KO, R], BF16, tag="yt")

                def attn_sub(g, hp, sub):
                    """S -> exp -> mask -> AV chain for one head, one row
                    group.  Returns the [65, 256] PSUM accumulator."""
                    jmax = 4 * g + 3
                    p0 = 64 * sub
                    ya = psy.tile([128, 256], F32, tag="ya")
                    chunks = list(range(jmax, -1, -1))
                    pts = []
                    for pi in range(len(chunks) // 2):
                        ja, jb = chunks[2 * pi], chunks[2 * pi + 1]
                        n0 = 128 * max(ja // 2 - 2 * g, 0)
                        N = 256 - n0
                        s2 = ps.tile([128, 2, 256], F32, tag="acc")
                        for jj, j in enumerate((ja, jb)):
                            nc.tensor.matmul(
                                s2[:, jj, 0:N],
                                kTg[p0:p0 + 64, hp, j * 128:(j + 1) * 128],
                                qT[p0:p0 + 64, hp,
                                   256 * g + n0:256 * g + 256],
                                start=(jj == 0), stop=(jj == 1))
                        pt = ptp.tile([128, 2, 256], BF16, tag="pt")
                        nc.scalar.activation(pt[:, :, 0:N], s2[:, :, 0:N],
                                             AF.Exp, scale=SCALE)
                        if pi < 2:
                            nc.vector.tensor_mul(
                                pt[:, :, 0:128], pt[:, :, 0:128],
                                mask_sb[:, 4 * g + 2 * pi:4 * g + 2 * pi + 2,
                                        :])
                        pts.append(pt)
                    for pi in range(len(chunks) // 2):
                        for jj in range(2):
                            j = chunks[2 * pi + jj]
                            n0 = 128 * max(j // 2 - 2 * g, 0)
                            nc.tensor.matmul(
                                ya[0:65, n0:256],
                                vext[:, j, 2 * hp + sub, :],
                                pts[pi][:, jj, 0:256 - n0],
                                start=(j == jmax), stop=(j == 0))
                    return ya

                def attn_norm(g, hp, sub, ya):
                    rl = act1.tile([1, 256], F32, tag="rl")
                    nc.vector.reciprocal(rl[:], ya[64:65, :])
                    rlb = act1.tile([64, 256], F32, tag="rlb")
                    nc.gpsimd.partition_broadcast(rlb[:], rl[:])
                    qc = 256 * g
                    if sub == 0:
                        nc.vector.tensor_tensor(yT2[0:64, hp, qc:qc + 256],
                                                ya[0:64, :], rlb[:], OP.mult)
                    else:
                        yodd = act1.tile([64, 256], BF16, tag="yodd")
                        nc.vector.tensor_tensor(yodd[:], ya[0:64, :],
                                                rlb[:], OP.mult)
                        nc.sync.dma_start(yT2[64:128, hp, qc:qc + 256],
                                          yodd[:])

                for g in range(2):
                    for hp in range(H // 2):
                        yas = [attn_sub(g, hp, sub) for sub in range(2)]
                        for sub in range(2):
                            attn_norm(g, hp, sub, yas[sub])

                # --- per group: proj, LN2, MLP (g1's attention overlaps
                # g0's proj/MLP via the scheduler) ---
                def proj_group(g):
                    for nch, (n0, nw) in enumerate(((0, 512), (512, 256))):
                        for rt in (2 * g, 2 * g + 1):
                            acc = ps.tile([128, 512], F32, tag="acc")
                            for hp in range(KO):
                                nc.tensor.matmul(
                                    acc[:, :nw],
                                    yT2[:, hp, rt * 128:(rt + 1) * 128],
                                    won[:, hp, n0:n0 + nw],
                                    start=(hp == 0), stop=False)
                            nc.tensor.matmul(acc[:, :nw], ones_r[:],
                                             bo_t[:, n0:n0 + nw],
                                             start=False, stop=True)
                            nc.vector.tensor_tensor(x_sb[:, rt, n0:n0 + nw],
                                                    x_sb[:, rt, n0:n0 + nw],
                                                    acc[:, :nw], OP.add)

                def mlp_group(g, hT2):
                    gts = []
                    for m in range(FF // 128):
                        w1m = wsmall.tile([128, KO, 128], BF16, tag="w1")
                        nc.sync.dma_start(
                            w1m[:],
                            w1_d.ap()[l, m].rearrange("p (ko j) -> p ko j",
                                                      ko=KO))
                        gacc = ps.tile([128, 512], F32, tag="acc")
                        for k in range(KO):
                            nc.tensor.matmul(
                                gacc[:, :256], w1m[:, k, :], hT2[:, k, :],
                                start=(k == 0), stop=(k == KO - 1))
                        gt_ = gtp.tile([128, 256], BF16, tag=f"g{m}")
                        nc.scalar.activation(gt_[:], gacc[:, :256], AF.Gelu,
                                             bias=b1T[:, m:m + 1])
                        gts.append(gt_)
                    for ri in range(2):
                        rt = 2 * g + ri
                        a = psf.tile([128, 512], F32, tag="fa")
                        b = psf.tile([128, 256], F32, tag="fb")
                        for m in range(FF // 128):
                            gsl = gts[m][:, ri * 128:(ri + 1) * 128]
                            nc.tensor.matmul(a[:], gsl, w2c[:, m, 0:512],
                                             start=(m == 0), stop=False)
                            nc.tensor.matmul(b[:], gsl, w2c[:, m, 512:768],
                                             start=(m == 0), stop=False)
                        for acc, n0, nw in ((a, 0, 512), (b, 512, 256)):
                            nc.tensor.matmul(acc[:, :nw], ones_r[:],
                                             b2_t[:, n0:n0 + nw],
                                             start=False, stop=True)
                            nc.vector.tensor_tensor(x_sb[:, rt, n0:n0 + nw],
                                                    x_sb[:, rt, n0:n0 + nw],
                                                    acc[:, :nw], OP.add)

                for g in range(2):
                    proj_group(g)
                    hT2 = layernorm_T(g2T, h2T, (2 * g, 2 * g + 1),
                                      f"ht2_{g}")
                    mlp_group(g, hT2)

            # ---- final LN + head ----
            gfT = cvec.tile([128, KO], F32, tag="g1T")
            nc.sync.dma_start(gfT[:], gf_d.ap().rearrange("(o p) -> p o", p=128))
            hfT = cvec.tile([128, KO], F32, tag="h1T")
            nc.sync.dma_start(hfT[:], hf_d.ap().rearrange("(o p) -> p o", p=128))
            xfT = layernorm_T(gfT, hfT, (0, 1, 2, 3), "ht")
            out_r = out_d.ap().rearrange("(o p) v -> p o v", p=128)
            for vs in range(1 if nohead else NVS):
                hwv = wmed.tile([128, KO, 512], BF16, tag="wv")
                nc.sync.dma_start(hwv[:],
                                  hw_d.ap()[:, :, vs * 512:(vs + 1) * 512])
                vw = min(512, V - vs * 512)
                for rt in range(RT):
                    acc = ps.tile([128, 512], F32, tag="acc")
                    for k in range(KO):
                        nc.tensor.matmul(acc[:_HEAD_M],
                                         xfT[:, k,
                                             rt * 128:rt * 128 + _HEAD_M],
                                         hwv[:, k, :],
                                         start=(k == 0), stop=(k == KO - 1))
                    st = stp.tile([128, 512], BF16, tag="lo")
                    nc.vector.tensor_copy(st[:], acc[:])
                    nc.sync.dma_start(
                        out_r[:, rt, vs * 512:vs * 512 + vw], st[:, :vw])

    nc.compile()
    return nc


def _prep_inputs(inputs):
    f = lambda k: np.asarray(inputs[k], dtype=np.float32)
    bf = lambda k: np.ascontiguousarray(
        np.asarray(inputs[k], dtype=np.float32)).astype(ml_dtypes.bfloat16)

    idx = np.asarray(inputs["idx"])
    tok = f("tok_emb")
    pos = f("pos_emb")[0]                      # [T, C]
    x0 = tok[idx] + pos[None, :, :]            # [B, T, C] f32

    hw = np.zeros((128, KO, VP), dtype=ml_dtypes.bfloat16)
    hw[:, :, :V] = bf("head_w").reshape(KO, 128, V).transpose(1, 0, 2)

    def pack_kT(w):            # [L, C, C] -> [L, KO(m), 128(p), (ko j)]
        a = w.reshape(L, KO, 128, KO, 128)         # (l, ko, p, m, j)
        return np.ascontiguousarray(a.transpose(0, 3, 2, 1, 4)).reshape(
            L, KO, 128, C)

    def pack_rhs(w, p):        # [L, K, N] -> [L, p, K//p(ko), N]
        ko = w.shape[1] // p
        a = w.reshape(L, ko, p, w.shape[2])
        return np.ascontiguousarray(a.transpose(0, 2, 1, 3))

    def pack_w1(w):            # [L, C, FF] -> [L, FF//128(m), 128(p), (ko j)]
        a = w.reshape(L, KO, 128, FF // 128, 128)  # (l, ko, p, m, j)
        return np.ascontiguousarray(a.transpose(0, 3, 2, 1, 4)).reshape(
            L, FF // 128, 128, C)

    shared = {
        "wq": pack_kT(bf("wq")), "wk": pack_kT(bf("wk")),
        "wv": pack_rhs(bf("wv"), 128), "wo": pack_rhs(bf("wo"), 128),
        "w1": pack_w1(bf("w1")), "w2": bf("w2"),
        "bq": f("bq"), "bk": f("bk"), "bv": bf("bv"), "bo": bf("bo"),
        "b1": f("b1"), "b2": bf("b2"),
        "ln1g": f("ln1_g"), "ln1b": f("ln1_b"),
        "ln2g": f("ln2_g"), "ln2b": f("ln2_b"),
        "lnfg": f("lnf_g"), "lnfb": f("lnf_b"),
        "headw": hw,
    }

    tril = (np.arange(128)[:, None] <= np.arange(128)[None, :])
    slots = [3, 2, 1, 0, 7, 6, 5, 4]           # descending-chunk order
    in_maps = []
    for core in range(NC_):
        b, s = core // 2, core % 2
        rows = np.concatenate(
            [np.arange((2 * p + s) * 128, (2 * p + s) * 128 + 128)
             for p in range(RT)])
        m = np.zeros((8, 128, 128), dtype=ml_dtypes.bfloat16)
        for si, j in enumerate(slots):
            if j % 2 == 0:
                m[si] = tril if s == 0 else 1.0
            else:
                m[si] = 0.0 if s == 0 else tril
        in_maps.append(dict(
            shared,
            x0=np.ascontiguousarray(x0[b, rows]),
            maskt=m,
        ))
    return in_maps


def kernel(**inputs):
    if "nc" not in _BUILD_CACHE:
        _BUILD_CACHE["nc"] = _build_nc()
    nc = _BUILD_CACHE["nc"]

    in_maps = _prep_inputs(inputs)
    res = bass_utils.run_bass_kernel_spmd(
        nc, in_maps, core_ids=list(range(NC_)))

    out = np.empty((B, T, V), dtype=np.float32)
    for core in range(NC_):
        b, s = core // 2, core % 2
        logits = res.results[core]["logits"].astype(np.float32)
        for p in range(RT):
            out[b, (2 * p + s) * 128:(2 * p + s + 1) * 128] = \
                logits[p * 128:(p + 1) * 128]
    return out


# revision 18
# speedup vs baseline: 6.7614x; 1.0531x over previous
"""GPT-2 (L=12, B=4, T=1024, C=768, H=12, V=50257) forward pass on 8 TRN2
NeuronCores.

Sharding: 8 cores = 4 sequences x 2 cores.  Core (b, s) owns the
INTERLEAVED query blocks {s, s+2, s+4, s+6} (128 rows each) of sequence b,
so both cores of a pair need the same causal chunk profile (2,4,6,8) per
local block and 37.5% of the S/AV/exp work is statically skipped.
Per-layer pairwise AllGather of (K^T, V-ext); gathered keys are assembled
in natural order.  Causal structure: key chunk j covers only the query
blocks that can see it (suffix of the local blocks); the two top chunks of
each block get a multiplicative [128x128] mask (tril / zeros / ones, a
per-core input).

Row groups g0 = blocks {0,1}, g1 = {2,3} are pipelined: attention of g1
(ACT-engine heavy) overlaps proj/LN2/MLP of g0 (PE heavy).

Compute dtype: bf16 matmuls with fp32 PSUM accumulation; fp32 residual
stream, layernorm stats, and softmax accumulation; bf16 logits.
"""

import numpy as np
import ml_dtypes

import concourse.bass as bass
import concourse.tile as tile
import concourse.mybir as mybir
from concourse import bacc, bass_utils
from concourse.masks import make_identity

F32 = mybir.dt.float32
BF16 = mybir.dt.bfloat16
AF = mybir.ActivationFunctionType
OP = mybir.AluOpType

L, B, T, C, H, V = 12, 4, 1024, 768, 12, 50257
D = C // H            # 64
FF = 4 * C            # 3072
R = 512               # rows per core
NC_ = 8               # cores
KO = C // 128         # 6
RT = R // 128         # 4 row tiles (local blocks)
NVS = 99              # vocab slices of 512
VP = NVS * 512        # 50688 padded vocab
SCALE = 1.0 / float(np.sqrt(D))
EPS = 1e-5

_BUILD_CACHE = {}
_WMED_BUFS = 2
_HEAD_M = 128


def _build_nc(sim=False, nocoll=False, nohead=False, nlayers=L):
    nc = bacc.Bacc("TRN2", target_bir_lowering=False, debug=False,
                   num_devices=NC_)

    # ---- I/O ----
    x0_d = nc.dram_tensor("x0", [R, C], F32, kind="ExternalInput")
    mask_d = nc.dram_tensor("maskt", [8, 128, 128], BF16, kind="ExternalInput")
    wq_d = nc.dram_tensor("wq", [L, KO, 128, C], BF16, kind="ExternalInput")
    wk_d = nc.dram_tensor("wk", [L, KO, 128, C], BF16, kind="ExternalInput")
    wv_d = nc.dram_tensor("wv", [L, 128, KO, C], BF16, kind="ExternalInput")
    wo_d = nc.dram_tensor("wo", [L, 128, KO, C], BF16, kind="ExternalInput")
    w1_d = nc.dram_tensor("w1", [L, FF // 128, 128, C], BF16, kind="ExternalInput")
    w2_d = nc.dram_tensor("w2", [L, FF, C], BF16, kind="ExternalInput")
    bq_d = nc.dram_tensor("bq", [L, C], F32, kind="ExternalInput")
    bk_d = nc.dram_tensor("bk", [L, C], F32, kind="ExternalInput")
    bv_d = nc.dram_tensor("bv", [L, C], BF16, kind="ExternalInput")
    bo_d = nc.dram_tensor("bo", [L, C], BF16, kind="ExternalInput")
    b1_d = nc.dram_tensor("b1", [L, FF], F32, kind="ExternalInput")
    b2_d = nc.dram_tensor("b2", [L, C], BF16, kind="ExternalInput")
    g1_d = nc.dram_tensor("ln1g", [L, C], F32, kind="ExternalInput")
    h1_d = nc.dram_tensor("ln1b", [L, C], F32, kind="ExternalInput")
    g2_d = nc.dram_tensor("ln2g", [L, C], F32, kind="ExternalInput")
    h2_d = nc.dram_tensor("ln2b", [L, C], F32, kind="ExternalInput")
    gf_d = nc.dram_tensor("lnfg", [C], F32, kind="ExternalInput")
    hf_d = nc.dram_tensor("lnfb", [C], F32, kind="ExternalInput")
    hw_d = nc.dram_tensor("headw", [NVS, 128, KO, 512], BF16, kind="ExternalInput")
    out_d = nc.dram_tensor("logits", [R, V], BF16, kind="ExternalOutput")

    KVLEN = C * R
    VLEN = RT * H * 65 * 128
    with tile.TileContext(nc) as tc:
        with (
            tc.tile_pool(name="const", bufs=1) as const,
            tc.tile_pool(name="persist", bufs=1) as persist,
            tc.tile_pool(name="wsmall", bufs=3) as wsmall,
            tc.tile_pool(name="wmed", bufs=_WMED_BUFS) as wmed,
            tc.tile_pool(name="wop", bufs=1) as wop,
            tc.tile_pool(name="wc", bufs=1) as wcp,
            tc.tile_pool(name="cvec", bufs=2) as cvec,
            tc.tile_pool(name="act1", bufs=2) as act1,
            tc.tile_pool(name="attn", bufs=1) as attnp,
            tc.tile_pool(name="pt", bufs=6) as ptp,
            tc.tile_pool(name="gt", bufs=1) as gtp,
            tc.tile_pool(name="st", bufs=3) as stp,
            tc.tile_pool(name="ps", bufs=3, space="PSUM") as ps,
            tc.tile_pool(name="psy", bufs=3, space="PSUM") as psy,
            tc.tile_pool(name="psf", bufs=1, space="PSUM") as psf,
            tc.tile_pool(name="dram", bufs=2, space="DRAM") as dramp,
        ):
            ident = const.tile([128, 128], BF16)
            make_identity(nc, ident[:])
            eps_t = const.tile([128, 1], F32)
            nc.vector.memset(eps_t[:], EPS)
            ones_r = const.tile([1, 128], BF16)
            nc.vector.memset(ones_r[:], 1.0)
            mask_sb = const.tile([128, 8, 128], BF16)
            nc.sync.dma_start(mask_sb[:], mask_d.ap().rearrange("j k q -> k j q"))

            # residual stream, fp32, [128, rt, C]; rt = local block
            x_sb = persist.tile([128, RT, C], F32)
            nc.sync.dma_start(
                x_sb[:], x0_d.ap().rearrange("(o p) c -> p o c", p=128))

            def layernorm_T(gT, bT, rts, tag):
                """LN(x[rts]) -> bf16, transposed into hT [128, KO, 128*len(rts)].

                gT/bT are [128, KO] partition-form gain/bias, applied after
                the transpose (feature dim lands on partitions there)."""
                hT = attnp.tile([128, KO, 128 * len(rts)], BF16, tag=tag)
                for ri, rt in enumerate(rts):
                    xs = x_sb[:, rt, :]
                    stats = act1.tile([128, 3, 6], F32, tag=f"st{tag}")
                    xs3 = xs.rearrange("p (s d) -> p s d", s=3)
                    for s in range(3):
                        nc.vector.bn_stats(stats[:, s, :], xs3[:, s, :])
                    mv = act1.tile([128, 2], F32, tag=f"mv{tag}")
                    nc.vector.bn_aggr(mv[:], stats[:])
                    std = act1.tile([128, 1], F32, tag=f"sd{tag}")
                    nc.scalar.activation(std[:], mv[:, 1:2], AF.Sqrt,
                                         bias=eps_t[:])
                    nc.vector.reciprocal(std[:], std[:])
                    hnb = act1.tile([128, C], BF16, tag=f"hb{tag}")
                    nc.vector.tensor_scalar(hnb[:], xs, mv[:, 0:1], std[:],
                                            op0=OP.subtract, op1=OP.mult)
                    for ko in range(KO):
                        pt_ = ps.tile([128, 512], BF16, tag="acc")
                        nc.tensor.transpose(pt_[:, :128],
                                            hnb[:, ko * 128:(ko + 1) * 128],
                                            ident[:])
                        nc.vector.tensor_scalar(
                            hT[:, ko, ri * 128:(ri + 1) * 128], pt_[:, :128],
                            gT[:, ko:ko + 1], bT[:, ko:ko + 1],
                            op0=OP.mult, op1=OP.add)
                return hT

            for l in range(nlayers):
                # --- per-layer constant vectors ---
                bqT = cvec.tile([128, KO], F32, tag="bqT")
                nc.sync.dma_start(bqT[:], bq_d.ap()[l].rearrange("(o p) -> p o", p=128))
                bkT = cvec.tile([128, KO], F32, tag="bkT")
                nc.sync.dma_start(bkT[:], bk_d.ap()[l].rearrange("(o p) -> p o", p=128))
                b1T = cvec.tile([128, FF // 128], F32, tag="b1T")
                nc.sync.dma_start(b1T[:], b1_d.ap()[l].rearrange("(o p) -> p o", p=128))
                g1T = cvec.tile([128, KO], F32, tag="g1T")
                nc.sync.dma_start(g1T[:], g1_d.ap()[l].rearrange("(o p) -> p o", p=128))
                h1T = cvec.tile([128, KO], F32, tag="h1T")
                nc.sync.dma_start(h1T[:], h1_d.ap()[l].rearrange("(o p) -> p o", p=128))
                g2T = cvec.tile([128, KO], F32, tag="g2T")
                nc.sync.dma_start(g2T[:], g2_d.ap()[l].rearrange("(o p) -> p o", p=128))
                h2T = cvec.tile([128, KO], F32, tag="h2T")
                nc.sync.dma_start(h2T[:], h2_d.ap()[l].rearrange("(o p) -> p o", p=128))
                bv_t = cvec.tile([1, C], BF16, tag="bv")
                nc.sync.dma_start(bv_t[:], bv_d.ap()[l][None, :])
                bo_t = cvec.tile([1, C], BF16, tag="bo")
                nc.sync.dma_start(bo_t[:], bo_d.ap()[l][None, :])
                b2_t = cvec.tile([1, C], BF16, tag="b2")
                nc.sync.dma_start(b2_t[:], b2_d.ap()[l][None, :])
                # prefetch the full-layer wo / w2 caches early
                won = wop.tile([128, KO, C], BF16, tag="wo")
                nc.sync.dma_start(won[:], wo_d.ap()[l])
                w2c = wcp.tile([128, FF // 128, C], BF16, tag="w2c")
                nc.sync.dma_start(
                    w2c[:], w2_d.ap()[l].rearrange("(m p) c -> p m c", p=128))

                # --- LN1 -> hT ---
                hT = layernorm_T(g1T, h1T, (0, 1, 2, 3), "ht")

                # --- kT = (h Wk)^T + bk ---  [128, KO, R]
                kT = attnp.tile([128, KO, R], BF16, tag="kt")
                for m in range(KO):
                    wkm = wsmall.tile([128, KO, 128], BF16, tag="wk")
                    nc.sync.dma_start(
                        wkm[:], wk_d.ap()[l, m].rearrange("p (ko j) -> p ko j", ko=KO))
                    acc = ps.tile([128, 512], F32, tag="acc")
                    for k in range(KO):
                        nc.tensor.matmul(acc[:], wkm[:, k, :], hT[:, k, :],
                                         start=(k == 0), stop=(k == KO - 1))
                    nc.vector.tensor_scalar_add(kT[:, m, :], acc[:],
                                                bkT[:, m:m + 1])
                k_in = dramp.tile([KVLEN], BF16, tag="kin")
                nc.sync.dma_start(
                    k_in[:].rearrange("(ko p r) -> p ko r", p=128, r=R),
                    kT[:])
                k_out = dramp.tile([2, KVLEN], BF16, tag="kout")
                if nocoll:
                    pass
                elif sim:
                    for rank in range(2):
                        nc.sync.dma_start(k_out[rank], k_in[:])
                else:
                    nc.gpsimd.collective_compute(
                        "AllGather", OP.bypass,
                        replica_groups=[[0, 1], [2, 3], [4, 5], [6, 7]],
                        ins=[k_in[:].opt()], outs=[k_out[:].opt()])

                # --- v = h Wv + bv ---  stored vext-shaped [128, RT, H, 65]
                v_sb = attnp.tile([128, RT, H, 65], BF16, tag="v")
                nc.vector.memset(v_sb[:, :, :, 64:65], 1.0)
                for nch, (n0, nw) in enumerate(((0, 512), (512, 256))):
                    wvn = wmed.tile([128, KO, 512], BF16, tag="wv")
                    nc.sync.dma_start(wvn[:, :, :nw],
                                      wv_d.ap()[l][:, :, n0:n0 + nw])
                    h0 = n0 // 64
                    for rt in range(RT):
                        acc = ps.tile([128, 512], F32, tag="acc")
                        for k in range(KO):
                            nc.tensor.matmul(
                                acc[:, :nw], hT[:, k, rt * 128:(rt + 1) * 128],
                                wvn[:, k, :nw],
                                start=(k == 0), stop=False)
                        nc.tensor.matmul(acc[:, :nw], ones_r[:],
                                         bv_t[:, n0:n0 + nw],
                                         start=False, stop=True)
                        nc.vector.tensor_copy(
                            v_sb[:, rt, h0:h0 + nw // 64, 0:64],
                            acc[:, :nw].rearrange("p (h d) -> p h d", d=64))
                v_in = dramp.tile([VLEN], BF16, tag="vin")
                nc.sync.dma_start(
                    v_in[:].rearrange("(o p x) -> p o x", p=128, x=H * 65),
                    v_sb[:])
                v_out = dramp.tile([2, VLEN], BF16, tag="vout")
                if nocoll:
                    pass
                elif sim:
                    for rank in range(2):
                        nc.sync.dma_start(v_out[rank], v_in[:])
                else:
                    nc.gpsimd.collective_compute(
                        "AllGather", OP.bypass,
                        replica_groups=[[0, 1], [2, 3], [4, 5], [6, 7]],
                        ins=[v_in[:].opt()], outs=[v_out[:].opt()])

                # --- qT = (h Wq)^T + bq ---
                qT = attnp.tile([128, KO, R], BF16, tag="qt")
                for m in range(KO):
                    wqm = wsmall.tile([128, KO, 128], BF16, tag="wq")
                    nc.sync.dma_start(
                        wqm[:], wq_d.ap()[l, m].rearrange("p (ko j) -> p ko j", ko=KO))
                    acc = ps.tile([128, 512], F32, tag="acc")
                    for k in range(KO):
                        nc.tensor.matmul(acc[:], wqm[:, k, :], hT[:, k, :],
                                         start=(k == 0), stop=(k == KO - 1))
                    nc.vector.tensor_scalar_add(qT[:, m, :], acc[:],
                                                bqT[:, m:m + 1])

                # --- assemble gathered kT / vext in natural key order ---
                # natural chunk j came from pair-member j%2, its block j//2
                kTg = attnp.tile([128, KO, T], BF16, tag="ktg")
                vext = attnp.tile([128, 8, H, 65], BF16, tag="vext")
                for j in range(8):
                    k_src = (k_in[:] if nocoll else k_out[j % 2])
                    v_src = (v_in[:] if nocoll else v_out[j % 2])
                    nc.sync.dma_start(
                        kTg[:, :, j * 128:(j + 1) * 128],
                        k_src
                        .rearrange("(ko p r) -> p ko r", p=128, r=R)
                        [:, :, (j // 2) * 128:(j // 2) * 128 + 128])
                    nc.sync.dma_start(
                        vext[:, j, :, :],
                        v_src
                        .rearrange("(o p h e) -> p o h e", p=128, h=H, e=65)
                        [:, j // 2])

                # --- attention, row-group pipelined; causal suffix-N chunks.
                # Chunk j covers local query blocks p >= j//2; within group g
                # (blocks 2g, 2g+1) its query range is [n0:256], n0 =
                # 128*max(j//2-2g, 0).  Chunks processed in descending j so
                # the per-element has_written bits make the suffix
                # accumulation correct.  The top two chunks of each block are
                # masked (tril/zeros/ones per core); mask slots are stored in
                # descending-chunk order [3,2,1,0,7,6,5,4]. ---
                yT2 = attnp.tile([128, KO, R], BF16, tag="yt")

                def attn_sub(g, hp, sub):
                    """S -> exp -> mask -> AV chain for one head, one row
                    group.  Returns the [65, 256] PSUM accumulator."""
                    jmax = 4 * g + 3
                    p0 = 64 * sub
                    ya = psy.tile([128, 256], F32, tag="ya")
                    chunks = list(range(jmax, -1, -1))
                    pts = []
                    for pi in range(len(chunks) // 2):
                        ja, jb = chunks[2 * pi], chunks[2 * pi + 1]
                        n0 = 128 * max(ja // 2 - 2 * g, 0)
                        N = 256 - n0
                        s2 = ps.tile([128, 2, 256], F32, tag="acc")
                        for jj, j in enumerate((ja, jb)):
                            nc.tensor.matmul(
                                s2[:, jj, 0:N],
                                kTg[p0:p0 + 64, hp, j * 128:(j + 1) * 128],
                                qT[p0:p0 + 64, hp,
                                   256 * g + n0:256 * g + 256],
                                start=(jj == 0), stop=(jj == 1))
                        pt = ptp.tile([128, 2, 256], BF16, tag="pt")
                        nc.scalar.activation(pt[:, :, 0:N], s2[:, :, 0:N],
                                             AF.Exp, scale=SCALE)
                        if pi < 2:
                            nc.vector.tensor_mul(
                                pt[:, :, 0:128], pt[:, :, 0:128],
                                mask_sb[:, 4 * g + 2 * pi:4 * g + 2 * pi + 2,
                                        :])
                        pts.append(pt)
                    for pi in range(len(chunks) // 2):
                        for jj in range(2):
                            j = chunks[2 * pi + jj]
                            n0 = 128 * max(j // 2 - 2 * g, 0)
                            nc.tensor.matmul(
                                ya[0:65, n0:256],
                                vext[:, j, 2 * hp + sub, :],
                                pts[pi][:, jj, 0:256 - n0],
                                start=(j == jmax), stop=(j == 0))
                    return ya

                def attn_norm(g, hp, sub, ya):
                    rl = act1.tile([1, 256], F32, tag="rl")
                    nc.vector.reciprocal(rl[:], ya[64:65, :])
                    rlb = act1.tile([64, 256], F32, tag="rlb")
                    nc.gpsimd.partition_broadcast(rlb[:], rl[:])
                    qc = 256 * g
                    if sub == 0:
                        nc.vector.tensor_tensor(yT2[0:64, hp, qc:qc + 256],
                                                ya[0:64, :], rlb[:], OP.mult)
                    else:
                        yodd = act1.tile([64, 256], BF16, tag="yodd")
                        nc.vector.tensor_tensor(yodd[:], ya[0:64, :],
                                                rlb[:], OP.mult)
                        nc.sync.dma_start(yT2[64:128, hp, qc:qc + 256],
                                          yodd[:])

                for g in range(2):
                    for hp in range(H // 2):
                        yas = [attn_sub(g, hp, sub) for sub in range(2)]
                        for sub in range(2):
                            attn_norm(g, hp, sub, yas[sub])

                # --- per group: proj, LN2, MLP (g1's attention overlaps
                # g0's proj/MLP via the scheduler) ---
                def proj_group(g):
                    for nch, (n0, nw) in enumerate(((0, 512), (512, 256))):
                        for rt in (2 * g, 2 * g + 1):
                            acc = ps.tile([128, 512], F32, tag="acc")
                            for hp in range(KO):
                                nc.tensor.matmul(
                                    acc[:, :nw],
                                    yT2[:, hp, rt * 128:(rt + 1) * 128],
                                    won[:, hp, n0:n0 + nw],
                                    start=(hp == 0), stop=False)
                            nc.tensor.matmul(acc[:, :nw], ones_r[:],
                                             bo_t[:, n0:n0 + nw],
                                             start=False, stop=True)
                            nc.vector.tensor_tensor(x_sb[:, rt, n0:n0 + nw],
                                                    x_sb[:, rt, n0:n0 + nw],
                                                    acc[:, :nw], OP.add)

                def mlp_group(g, hT2):
                    gts = []
                    for m in range(FF // 128):
                        w1m = wsmall.tile([128, KO, 128], BF16, tag="w1")
                        nc.sync.dma_start(
                            w1m[:],
                            w1_d.ap()[l, m].rearrange("p (ko j) -> p ko j",
                                                      ko=KO))
                        gacc = ps.tile([128, 512], F32, tag="acc")
                        for k in range(KO):
                            nc.tensor.matmul(
                                gacc[:, :256], w1m[:, k, :], hT2[:, k, :],
                                start=(k == 0), stop=(k == KO - 1))
                        gt_ = gtp.tile([128, 256], BF16, tag=f"g{m}")
                        nc.scalar.activation(gt_[:], gacc[:, :256], AF.Gelu,
                                             bias=b1T[:, m:m + 1])
                        gts.append(gt_)
                    for ri in range(2):
                        rt = 2 * g + ri
                        a = psf.tile([128, 512], F32, tag="fa")
                        b = psf.tile([128, 256], F32, tag="fb")
                        for m in range(FF // 128):
                            gsl = gts[m][:, ri * 128:(ri + 1) * 128]
                            nc.tensor.matmul(a[:], gsl, w2c[:, m, 0:512],
                                             start=(m == 0), stop=False)
                            nc.tensor.matmul(b[:], gsl, w2c[:, m, 512:768],
                                             start=(m == 0), stop=False)
                        for acc, n0, nw in ((a, 0, 512), (b, 512, 256)):
                            nc.tensor.matmul(acc[:, :nw], ones_r[:],
                                             b2_t[:, n0:n0 + nw],
                                             start=False, stop=True)
                            nc.vector.tensor_tensor(x_sb[:, rt, n0:n0 + nw],
                                                    x_sb[:, rt, n0:n0 + nw],
                                                    acc[:, :nw], OP.add)

                for g in range(2):
                    proj_group(g)
                    hT2 = layernorm_T(g2T, h2T, (2 * g, 2 * g + 1),
                                      f"ht2_{g}")
                    mlp_group(g, hT2)

            # ---- final LN + head ----
            gfT = cvec.tile([128, KO], F32, tag="g1T")
            nc.sync.dma_start(gfT[:], gf_d.ap().rearrange("(o p) -> p o", p=128))
            hfT = cvec.tile([128, KO], F32, tag="h1T")
            nc.sync.dma_start(hfT[:], hf_d.ap().rearrange("(o p) -> p o", p=128))
            xfT = layernorm_T(gfT, hfT, (0, 1, 2, 3), "ht")
            out_r = out_d.ap().rearrange("(o p) v -> p o v", p=128)
            for vs in range(1 if nohead else NVS):
                hwv = wmed.tile([128, KO, 512], BF16, tag="wv")
                nc.sync.dma_start(hwv[:], hw_d.ap()[vs])
                vw = min(512, V - vs * 512)
                st4 = stp.tile([128, RT, 512], BF16, tag="lo")
                for rt in range(RT):
                    acc = ps.tile([128, 512], F32, tag="acc")
                    for k in range(KO):
                        nc.tensor.matmul(acc[:_HEAD_M],
                                         xfT[:, k,
                                             rt * 128:rt * 128 + _HEAD_M],
                                         hwv[:, k, :],
                                         start=(k == 0), stop=(k == KO - 1))
                    nc.vector.tensor_copy(st4[:, rt, :], acc[:])
                nc.sync.dma_start(
                    out_r[:, :, vs * 512:vs * 512 + vw], st4[:, :, :vw])

    nc.compile()
    return nc


def _prep_inputs(inputs):
    f = lambda k: np.asarray(inputs[k], dtype=np.float32)
    bf = lambda k: np.ascontiguousarray(
        np.asarray(inputs[k], dtype=np.float32)).astype(ml_dtypes.bfloat16)

    idx = np.asarray(inputs["idx"])
    tok = f("tok_emb")
    pos = f("pos_emb")[0]                      # [T, C]
    x0 = tok[idx] + pos[None, :, :]            # [B, T, C] f32

    hw = np.zeros((128, KO, VP), dtype=ml_dtypes.bfloat16)
    hw[:, :, :V] = bf("head_w").reshape(KO, 128, V).transpose(1, 0, 2)
    hw = np.ascontiguousarray(
        hw.reshape(128, KO, NVS, 512).transpose(2, 0, 1, 3))

    def pack_kT(w):            # [L, C, C] -> [L, KO(m), 128(p), (ko j)]
        a = w.reshape(L, KO, 128, KO, 128)         # (l, ko, p, m, j)
        return np.ascontiguousarray(a.transpose(0, 3, 2, 1, 4)).reshape(
            L, KO, 128, C)

    def pack_rhs(w, p):        # [L, K, N] -> [L, p, K//p(ko), N]
        ko = w.shape[1] // p
        a = w.reshape(L, ko, p, w.shape[2])
        return np.ascontiguousarray(a.transpose(0, 2, 1, 3))

    def pack_w1(w):            # [L, C, FF] -> [L, FF//128(m), 128(p), (ko j)]
        a = w.reshape(L, KO, 128, FF // 128, 128)  # (l, ko, p, m, j)
        return np.ascontiguousarray(a.transpose(0, 3, 2, 1, 4)).reshape(
            L, FF // 128, 128, C)

    shared = {
        "wq": pack_kT(bf("wq")), "wk": pack_kT(bf("wk")),
        "wv": pack_rhs(bf("wv"), 128), "wo": pack_rhs(bf("wo"), 128),
        "w1": pack_w1(bf("w1")), "w2": bf("w2"),
        "bq": f("bq"), "bk": f("bk"), "bv": bf("bv"), "bo": bf("bo"),
        "b1": f("b1"), "b2": bf("b2"),
        "ln1g": f("ln1_g"), "ln1b": f("ln1_b"),
        "ln2g": f("ln2_g"), "ln2b": f("ln2_b"),
        "lnfg": f("lnf_g"), "lnfb": f("lnf_b"),
        "headw": hw,
    }

    tril = (np.arange(128)[:, None] <= np.arange(128)[None, :])
    slots = [3, 2, 1, 0, 7, 6, 5, 4]           # descending-chunk order
    in_maps = []
    for core in range(NC_):
        b, s = core // 2, core % 2
        rows = np.concatenate(
            [np.arange((2 * p + s) * 128, (2 * p + s) * 128 + 128)
             for p in range(RT)])
        m = np.zeros((8, 128, 128), dtype=ml_dtypes.bfloat16)
        for si, j in enumerate(slots):
            if j % 2 == 0:
                m[si] = tril if s == 0 else 1.0
            else:
                m[si] = 0.0 if s == 0 else tril
        in_maps.append(dict(
            shared,
            x0=np.ascontiguousarray(x0[b, rows]),
            maskt=m,
        ))
    return in_maps


def kernel(**inputs):
    if "nc" not in _BUILD_CACHE:
        _BUILD_CACHE["nc"] = _build_nc()
    nc = _BUILD_CACHE["nc"]

    in_maps = _prep_inputs(inputs)
    res = bass_utils.run_bass_kernel_spmd(
        nc, in_maps, core_ids=list(range(NC_)))

    out = np.empty((B, T, V), dtype=np.float32)
    for core in range(NC_):
        b, s = core // 2, core % 2
        logits = res.results[core]["logits"].astype(np.float32)
        for p in range(RT):
            out[b, (2 * p + s) * 128:(2 * p + s + 1) * 128] = \
                logits[p * 128:(p + 1) * 128]
    return out
